# revision 1
# baseline (speedup 1.0000x reference)
"""BERT self-attention on 8 TRN2 NeuronCores.

Problem: hidden_states [4, 2048, 1024], 16 heads x 64 dim, fp32.
Sharding: core c handles batch b = c//2 and head-group g = c%2
(8 heads = 512 embedding columns per core). Full inputs in, full
output out; slicing/transposition of inputs happens host-side here.

Per-core device kernel (all f32r matmuls = fp32 bits, TF32-like):
  Phase 1: Q^T/K^T [e,s] and V [s,e] projections from X^T [h,s],
           weights pre-transposed host-side. V gets a ones column
           appended per head (denominator trick) and bias via a
           K=1 matmul row; Q/K biases via per-partition DVE add.
  Phase 2: per head: S^T[k,q] = K^T.T @ Q^T (K=64 contraction),
           expS = exp(S^T*0.125 + mask_k) on ACT (mask is a
           per-partition bias in this layout), ctx'[d|denom, q]
           accumulated over k-tiles with lhsT = V'[k, 65].
           Epilogue: PE-transpose 128-blocks -> [q, 65], DVE
           reciprocal of col 64, per-partition scale of cols 0:64,
           DMA to out.
"""

import os
import numpy as np

import concourse.bass as bass
import concourse.tile as tile
from concourse import bacc, mybir
from concourse.bass_utils import run_bass_kernel_spmd
from concourse.masks import make_identity

F32 = mybir.dt.float32
F32R = mybir.dt.float32r

B, S, H = 4, 2048, 1024
NH, HD = 16, 64
NCORES = 8
E = 512          # embedding columns per core (8 heads)
NHL = 8          # heads per core
NKT = S // 128   # 16 k-tiles
NET = E // 128   # 4 e-tiles
NHT = H // 128   # 8 h-tiles
QW = 512         # per-head q-chunk width (exp ops span head pairs: 1024)

_CACHE = {}

KERNEL_VERSION = "v13"  # bump to bust the neuron compile cache on kernel changes

LAST_PROFILE = {}


def build_kernel(with_vbias=True):
    nc = bacc.Bacc("TRN2", target_bir_lowering=False, debug=False,
                   num_devices=NCORES)

    xt = nc.dram_tensor("xt", [H, S], F32R, kind="ExternalInput").ap()
    wqt = nc.dram_tensor("wqt", [H, E], F32R, kind="ExternalInput").ap()
    wkt = nc.dram_tensor("wkt", [H, E], F32R, kind="ExternalInput").ap()
    wvt = nc.dram_tensor("wvt", [H, E], F32R, kind="ExternalInput").ap()
    bq2 = nc.dram_tensor("bq2", [128, NET], F32, kind="ExternalInput").ap()
    bk2 = nc.dram_tensor("bk2", [128, NET], F32, kind="ExternalInput").ap()
    bv2 = nc.dram_tensor("bv2", [1, E], F32R, kind="ExternalInput").ap()
    suffix = f"{KERNEL_VERSION}{'b' if with_vbias else ''}"
    mask2 = nc.dram_tensor(f"mask2_{suffix}", [128, NKT], F32,
                           kind="ExternalInput").ap()
    out = nc.dram_tensor("out", [S, E], F32, kind="ExternalOutput").ap()

    Exp = mybir.ActivationFunctionType.Exp

    with tile.TileContext(nc) as tc:
        with (
            tc.tile_pool(name="persist", bufs=1) as persist,
            tc.tile_pool(name="small", bufs=1) as small,
        ):
            # persistent SBUF tensors, split per chunk so each has a
            # single producer -> exact dependencies, phases can overlap
            qt_t = [[persist.tile([128, 512], F32R, name=f"qt_{et}_{sc}")
                     for sc in range(4)] for et in range(NET)]
            kt_t = [[persist.tile([128, 512], F32R, name=f"kt_{et}_{sc}")
                     for sc in range(4)] for et in range(NET)]
            vp_t = [persist.tile([128, NHL * 65], F32R, name=f"vp_{gst}")
                    for gst in range(NKT)]

            mask_sb = small.tile([128, NKT], F32)
            nc.sync.dma_start(mask_sb[:], mask2)
            bq_sb = small.tile([128, NET], F32)
            nc.sync.dma_start(bq_sb[:], bq2)
            bk_sb = small.tile([128, NET], F32)
            nc.sync.dma_start(bk_sb[:], bk2)
            bv_sb = small.tile([1, E], F32R)
            nc.sync.dma_start(bv_sb[:], bv2)
            ones_f = small.tile([128, 128], F32)
            nc.vector.memset(ones_f[:], 1.0)
            ones_row = small.tile([1, 128], F32R)
            nc.vector.tensor_copy(ones_row[:], ones_f[0:1, :])
            ident = small.tile([128, 128], F32)
            make_identity(nc, ident[:])

            # ones columns of V' (denominator trick)
            for gst in range(NKT):
                vcols = vp_t[gst].rearrange("p (t c) -> p t c", c=65)
                nc.vector.tensor_copy(
                    vcols[:, :, 64:65],
                    ones_f[:, 0:NHL].rearrange("p (t c) -> p t c", c=1))

            # ---- unified pools (phases overlap at runtime) ----
            # PSUM banks: ss(proj+scores) 2 slots x 2 banks = 4,
            #             cp 2 x 1 = 2, tp 2 x 1 = 2  -> 8 total
            with (
                tc.tile_pool(name="xtp", bufs=16) as xtp,
                tc.tile_pool(name="wp", bufs=1) as wp,
                tc.tile_pool(name="ssp", bufs=2, space="PSUM") as ssp,
                tc.tile_pool(name="cpp", bufs=2, space="PSUM") as cpp,
                tc.tile_pool(name="tpp", bufs=2, space="PSUM") as tpp,
                tc.tile_pool(name="exp", bufs=4) as ex_pool,
                tc.tile_pool(name="csp", bufs=3) as cs_pool,
                tc.tile_pool(name="obp", bufs=2) as ob_pool,
                tc.tile_pool(name="rcp", bufs=4) as rc_pool,
            ):
                # ---------------- Phase 1: projections ----------------
                wq_t, wk_t, wv_t = [], [], []
                xts0 = []
                for h in range(NHT):
                    t = xtp.tile([128, 512], F32R, tag="xt", name=f"xt0_{h}")
                    nc.sync.dma_start(t[:], xt[h * 128:(h + 1) * 128, 0:512])
                    xts0.append(t)
                    w = wp.tile([128, E], F32R, name=f"wk_{h}")
                    nc.sync.dma_start(w[:], wkt[h * 128:(h + 1) * 128, :])
                    wk_t.append(w)
                for h in range(NHT):
                    w = wp.tile([128, E], F32R, name=f"wv_{h}")
                    nc.sync.dma_start(w[:], wvt[h * 128:(h + 1) * 128, :])
                    wv_t.append(w)
                for h in range(NHT):
                    w = wp.tile([128, E], F32R, name=f"wq_{h}")
                    nc.sync.dma_start(w[:], wqt[h * 128:(h + 1) * 128, :])
                    wq_t.append(w)

                p2state = {}

                def p2_kloop(et, qc, kts):
                    hA, hB = 2 * et, 2 * et + 1
                    q0 = qc * QW
                    key = (et, qc)
                    if key not in p2state:
                        cpA = cpp.tile([65, QW], F32, tag="cp",
                                       name=f"cpA_{et}_{qc}")
                        cpB = cpp.tile([65, QW], F32, tag="cp",
                                       name=f"cpB_{et}_{qc}")
                        p2state[key] = (cpA, cpB)
                    cpA, cpB = p2state[key]
                    for kt in kts:
                        sps = ssp.tile([128, 2 * QW], F32, tag="ss",
                                       name=f"sps_{et}_{qc}_{kt}")
                        ktt = kt_t[et][kt // 4]
                        qtt = qt_t[et][qc]
                        ko = (kt % 4) * 128
                        nc.tensor.matmul(
                            sps[:, 0:QW],
                            ktt[0:64, ko:ko + 128],
                            qtt[0:64, :], start=True, stop=True)
                        nc.tensor.matmul(
                            sps[:, QW:2 * QW],
                            ktt[64:128, ko:ko + 128],
                            qtt[64:128, :], start=True, stop=True)
                        ex = ex_pool.tile([128, 2 * QW], F32R, tag="ex",
                                          name=f"ex_{et}_{qc}_{kt}")
                        nc.scalar.activation(
                            ex[:], sps[:], Exp,
                            bias=mask_sb[:, kt:kt + 1], scale=0.125)
                        nc.tensor.matmul(
                            cpA[:],
                            vp_t[kt][:, hA * 65:hA * 65 + 65],
                            ex[:, 0:QW],
                            start=(kt == 0), stop=(kt == NKT - 1))
                        nc.tensor.matmul(
                            cpB[:],
                            vp_t[kt][:, hB * 65:hB * 65 + 65],
                            ex[:, QW:2 * QW],
                            start=(kt == 0), stop=(kt == NKT - 1))

                def p2_epilogue(et, qc):
                    hA, hB = 2 * et, 2 * et + 1
                    q0 = qc * QW
                    cpA, cpB = p2state.pop((et, qc))
                    for (hl, cpx) in ((hA, cpA), (hB, cpB)):
                        cts = cs_pool.tile([65, QW], F32, tag="cs",
                                           name=f"cts_{et}_{qc}_{hl}")
                        nc.vector.tensor_copy(cts[:], cpx[:])
                        obig = ob_pool.tile([128, (QW // 128) * 64], F32,
                                            tag="ob", name=f"ob_{et}_{qc}_{hl}")
                        for i in range(QW // 128):
                            tp = tpp.tile([128, 65], F32, tag="tp",
                                          name=f"tp_{et}_{qc}_{hl}_{i}")
                            nc.tensor.transpose(
                                tp[:], cts[:, i * 128:(i + 1) * 128],
                                ident[0:65, 0:65])
                            rc = rc_pool.tile([128, 1], F32, tag="rc",
                                              name=f"rc_{et}_{qc}_{hl}_{i}")
                            nc.vector.reciprocal(rc[:], tp[:, 64:65])
                            nc.vector.tensor_scalar_mul(
                                obig[:, i * 64:(i + 1) * 64],
                                tp[:, 0:64], rc[:])
                        od = out[q0:q0 + QW, hl * 64:(hl + 1) * 64]
                        od = od.rearrange("(i p) c -> p i c", p=128)
                        nc.sync.dma_start(
                            od, obig.rearrange("p (i c) -> p i c", c=64))

                def proj_qk(w_t, b_sb, dst, xts, s0):
                    sc = s0 // 512
                    for et in range(NET):
                        p = ssp.tile([128, 512], F32, tag="ss",
                                     name=f"pj_{s0}_{et}")
                        for h in range(NHT):
                            nc.tensor.matmul(
                                p[:],
                                w_t[h][:, et * 128:et * 128 + 128],
                                xts[h][:],
                                start=(h == 0), stop=(h == NHT - 1))
                        nc.vector.tensor_scalar_add(
                            dst[et][sc][:], p[:], b_sb[:, et:et + 1])

                all_xts = {}
                for sc in range(4):  # s-chunks of 512
                    s0 = sc * 512
                    if sc == 0:
                        xts = xts0
                    else:
                        xts = []
                        for h in range(NHT):
                            t = xtp.tile([128, 512], F32R, tag="xt",
                                         name=f"xt{sc}_{h}")
                            nc.sync.dma_start(t[:], xt[h * 128:(h + 1) * 128,
                                                       s0:s0 + 512])
                            xts.append(t)
                    all_xts[sc] = xts

                    proj_qk(wk_t, bk_sb, kt_t, xts, s0)

                    # V s-tiles: out [s 128, e 512] (+ bias via K=1 matmul)
                    for st in range(4):
                        gst = sc * 4 + st
                        p = ssp.tile([128, 512], F32, tag="ss")
                        for h in range(NHT):
                            nc.tensor.matmul(
                                p[:],
                                xts[h][:, st * 128:st * 128 + 128],
                                wv_t[h][:],
                                start=(h == 0),
                                stop=(not with_vbias and h == NHT - 1))
                        if with_vbias:
                            nc.tensor.matmul(p[:], ones_row[:], bv_sb[:],
                                             start=False, stop=True)
                        dstv = vp_t[gst].rearrange("p (h c) -> p h c", c=65)
                        nc.vector.tensor_copy(
                            dstv[:, :, 0:64],
                            p.rearrange("p (h c) -> p h c", c=64))

                    # Q for early q-chunks only; chunks 2,3 are deferred into
                    # phase 2 where PE micro-gaps absorb them
                    if sc < 2:
                        proj_qk(wq_t, bq_sb, qt_t, xts, s0)

                    # head-pair 0 / q-chunk 0: this chunk's k-tiles are ready
                    # now -> feeds ACT during phase 1
                    p2_kloop(0, 0, range(4 * sc, 4 * sc + 4))


                # ---------------- Phase 2: attention (head pairs) -------
                p2_epilogue(0, 0)
                deferred_q = [2, 3]
                for et in range(NET):
                    for qc in range(S // QW):
                        if et == 0 and deferred_q and qc == deferred_q[0] - 2:
                            sc = deferred_q.pop(0)
                            proj_qk(wq_t, bq_sb, qt_t, all_xts[sc],
                                    sc * 512)
                        if (et, qc) == (0, 0):
                            continue
                        p2_kloop(et, qc, range(NKT))
                        p2_epilogue(et, qc)

    nc.compile()
    return nc


def build_in_maps(inputs, with_vbias=None):
    if with_vbias is None:
        with_vbias = bool(np.any(np.asarray(inputs["bv"], np.float32)))
    vb = "b" if with_vbias else ""
    hidden_states = np.asarray(inputs["hidden_states"], dtype=np.float32)
    attention_mask = np.asarray(inputs["attention_mask"], dtype=np.float32)
    Wq, bq = np.asarray(inputs["Wq"], np.float32), np.asarray(inputs["bq"], np.float32)
    Wk, bk = np.asarray(inputs["Wk"], np.float32), np.asarray(inputs["bk"], np.float32)
    Wv, bv = np.asarray(inputs["Wv"], np.float32), np.asarray(inputs["bv"], np.float32)

    xts = [np.ascontiguousarray(hidden_states[b].T) for b in range(B)]
    masks = [np.ascontiguousarray(attention_mask[b, 0, 0].reshape(NKT, 128).T)
             for b in range(B)]
    wg = []
    for g in range(2):
        rows = slice(g * E, (g + 1) * E)
        wg.append({
            "wqt": np.ascontiguousarray(Wq[rows].T),
            "wkt": np.ascontiguousarray(Wk[rows].T),
            "wvt": np.ascontiguousarray(Wv[rows].T),
            "bq2": np.ascontiguousarray(bq[rows].reshape(NET, 128).T),
            "bk2": np.ascontiguousarray(bk[rows].reshape(NET, 128).T),
            "bv2": np.ascontiguousarray(bv[rows].reshape(1, E)),
        })
    in_maps = []
    for c in range(NCORES):
        b, g = c // 2, c % 2
        in_maps.append({
            "xt": xts[b],
            f"mask2_{KERNEL_VERSION}{vb}": masks[b],
            **wg[g],
        })
    return in_maps


def kernel(hidden_states, attention_mask, Wq, bq, Wk, bk, Wv, bv):
    with_vbias = bool(np.any(np.asarray(bv, np.float32)))
    ckey = ("nc", with_vbias)
    if ckey not in _CACHE:
        _CACHE[ckey] = build_kernel(with_vbias)
    nc = _CACHE[ckey]

    in_maps = build_in_maps(dict(
        hidden_states=hidden_states, attention_mask=attention_mask,
        Wq=Wq, bq=bq, Wk=Wk, bk=bk, Wv=Wv, bv=bv))

    trace = bool(int(os.environ.get("BASS_KERNEL_TRACE", "0")))
    res = run_bass_kernel_spmd(nc, in_maps, core_ids=list(range(NCORES)),
                               trace=trace)
    LAST_PROFILE["exec_time_ns"] = res.exec_time_ns
    LAST_PROFILE["mean_exec_time_ns"] = res.mean_exec_time_ns
    if res.instructions_and_trace is not None:
        LAST_PROFILE["trace_path"] = res.instructions_and_trace[1]

    full = np.empty((B, S, H), dtype=np.float32)
    for c in range(NCORES):
        b, g = c // 2, c % 2
        full[b][:, g * E:(g + 1) * E] = res.results[c]["out"]
    return full



# revision 17
# speedup vs baseline: 1.2672x; 1.2672x over previous
"""BERT self-attention on 8 TRN2 NeuronCores.

Problem: hidden_states [4, 2048, 1024], 16 heads x 64 dim, fp32.
Sharding: core c handles batch b = c//2 and head-group g = c%2
(8 heads = 512 embedding columns per core). Full inputs in, full
output out; slicing/transposition of inputs happens host-side here.

v14 design (per-core):
  All matmul operands fp16 (error budget allows: rel err ~8e-3 vs
  the 2e-2 gate); PSUM accumulation fp32.
  Phase 1: Q^T/K^T [e,s] and V [s,e] projections from X^T [h,s],
           weights pre-transposed host-side, all fp16. Q/K biases via
           DVE add on the PSUM->SBUF copy; V bias (when nonzero) via a
           K=1 ones-row matmul. V' gets a ones column per head
           (softmax denominator trick) via a tile-wide memset to 1.0
           before the 64-col blocks are copied in.
  Phase 2: per (head-pair et, q-chunk qc of 512):
           S^T[k,q] = K^T.T @ Q^T (K=64 contraction, N=512 moving),
           expS = exp(S*0.125 + mask_k): most k-tiles on ACT (true
           exp -> fp16), a tunable subset on DVE via the Schraudolph
           trick (bits = A*arg + B as int16, bitcast to fp16; ~3% rel
           err, consistent numerator/denominator so it mostly cancels).
           ctx: queries in the PE partition dim: cp[128q, 4j x 65] +=
           ex[k, q-tile].T @ V'[k, 65] -- N=65 per matmul instead of
           512, 2x fewer PE cycles than the [65, q] orientation, and
           the [q, 65] output needs no PE transpose in the epilogue.
           Epilogue: DVE reciprocal of col 64, per-partition scale of
           cols 0:64, one [128, 4, 128] DMA per (et, qc) to out.
"""

import os
import numpy as np

import concourse.bass as bass
import concourse.tile as tile
from concourse import bacc, mybir
from concourse.bass_utils import run_bass_kernel_spmd

F32 = mybir.dt.float32
F16 = mybir.dt.float16
I16 = mybir.dt.int16

B, S, H = 4, 2048, 1024
NH, HD = 16, 64
NCORES = 8
E = 512          # embedding columns per core (8 heads)
NHL = 8          # heads per core
NKT = S // 128   # 16 k-tiles
NET = E // 128   # 4 e-tiles (head pairs)
NHT = H // 128   # 8 h-tiles
QW = 512         # per-head q-chunk width

# Schraudolph fast-exp constants (fp16 bits = round(A*arg + B16))
SCH_A = 1024.0 / float(np.log(2.0))
SCH_B = 1024.0 * 15 - 44.25
# Per (k-tile, head) the scores land in their own 1-bank PSUM tile and
# the exp runs as one full-tile instruction: one head on ACT (true exp),
# the other on DVE (Schraudolph bits trick), swapping per k-tile parity.
# Both engines run concurrently, halving the exp latency in the
# scores->exp->ctx chain; every softmax row is 50/50 exact/approximate.

_CACHE = {}

KERNEL_VERSION = "v14"  # bump to bust the neuron compile cache on kernel changes

LAST_PROFILE = {}


def build_kernel(with_vbias=True, with_sch=True):
    nc = bacc.Bacc("TRN2", target_bir_lowering=False, debug=False,
                   num_devices=NCORES)

    xt = nc.dram_tensor("xt", [H, S], F16, kind="ExternalInput").ap()
    wqt = nc.dram_tensor("wqt", [H, E], F16, kind="ExternalInput").ap()
    wkt = nc.dram_tensor("wkt", [H, E], F16, kind="ExternalInput").ap()
    wvt = nc.dram_tensor("wvt", [H, E], F16, kind="ExternalInput").ap()
    bq2 = nc.dram_tensor("bq2", [128, NET], F32, kind="ExternalInput").ap()
    bk2 = nc.dram_tensor("bk2", [128, NET], F32, kind="ExternalInput").ap()
    bv2 = nc.dram_tensor("bv2", [1, E], F16, kind="ExternalInput").ap()
    suffix = f"{KERNEL_VERSION}{'b' if with_vbias else ''}{'s' if with_sch else ''}"
    mask2 = nc.dram_tensor(f"mask2_{suffix}", [128, NKT], F32,
                           kind="ExternalInput").ap()
    smask2 = nc.dram_tensor("smask2", [128, NKT], F32,
                            kind="ExternalInput").ap()
    out = nc.dram_tensor("out", [S, E], F32, kind="ExternalOutput").ap()

    Exp = mybir.ActivationFunctionType.Exp
    Mult, Add = mybir.AluOpType.mult, mybir.AluOpType.add

    with tile.TileContext(nc) as tc:
        with (
            tc.tile_pool(name="persist", bufs=1) as persist,
            tc.tile_pool(name="small", bufs=1) as small,
        ):
            # persistent SBUF tensors, split per chunk so each has a
            # single producer -> exact dependencies, phases can overlap
            qt_t = [[persist.tile([128, 512], F16, name=f"qt_{et}_{sc}")
                     for sc in range(4)] for et in range(NET)]
            kt_t = [[persist.tile([128, 512], F16, name=f"kt_{et}_{sc}")
                     for sc in range(4)] for et in range(NET)]
            vp_t = [persist.tile([128, NHL * 65], F16, name=f"vp_{gst}")
                    for gst in range(NKT)]

            mask_sb = small.tile([128, NKT], F32)
            nc.sync.dma_start(mask_sb[:], mask2)
            smask_sb = small.tile([128, NKT], F32)
            nc.sync.dma_start(smask_sb[:], smask2)
            bq_sb = small.tile([128, NET], F32)
            nc.sync.dma_start(bq_sb[:], bq2)
            bk_sb = small.tile([128, NET], F32)
            nc.sync.dma_start(bk_sb[:], bk2)
            bv_sb = small.tile([1, E], F16)
            nc.sync.dma_start(bv_sb[:], bv2)
            ones_row = small.tile([1, 128], F16)
            nc.vector.memset(ones_row[:], 1.0)

            # ones columns of V' (denominator trick): memset whole tile
            # to 1.0; the V-projection copies later overwrite cols 0:64
            # of each head's 65-block, leaving col 64 = 1.0.
            for gst in range(NKT):
                nc.vector.memset(vp_t[gst][:], 1.0)

            # ---- unified pools (phases overlap at runtime) ----
            # PSUM banks: ss(proj+scores) 4 slots x 1 bank ([128,512] f32),
            #             cp 4 x 1 bank ([128, 260] f32) -> 8 total
            with (
                tc.tile_pool(name="xtp", bufs=16) as xtp,
                tc.tile_pool(name="wp", bufs=1) as wp,
                tc.tile_pool(name="ssp", bufs=4, space="PSUM") as ssp,
                tc.tile_pool(name="cpp", bufs=4, space="PSUM") as cpp,
                tc.tile_pool(name="exa", bufs=4) as exa_pool,
                tc.tile_pool(name="obp", bufs=2) as ob_pool,
                tc.tile_pool(name="rcp", bufs=4) as rc_pool,
            ):
                # ---------------- Phase 1: projections ----------------
                wq_t, wk_t, wv_t = [], [], []
                xts0 = []
                for h in range(NHT):
                    t = xtp.tile([128, 512], F16, tag="xt", name=f"xt0_{h}")
                    nc.sync.dma_start(t[:], xt[h * 128:(h + 1) * 128, 0:512])
                    xts0.append(t)
                    w = wp.tile([128, E], F16, name=f"wk_{h}")
                    nc.sync.dma_start(w[:], wkt[h * 128:(h + 1) * 128, :])
                    wk_t.append(w)
                for h in range(NHT):
                    w = wp.tile([128, E], F16, name=f"wv_{h}")
                    nc.sync.dma_start(w[:], wvt[h * 128:(h + 1) * 128, :])
                    wv_t.append(w)
                for h in range(NHT):
                    w = wp.tile([128, E], F16, name=f"wq_{h}")
                    nc.sync.dma_start(w[:], wqt[h * 128:(h + 1) * 128, :])
                    wq_t.append(w)

                p2state = {}
                # software pipeline: ctx(kt) is emitted ~CTX_LAG k-steps
                # after its scores/exp, so the in-order PE always has the
                # next scores ready while ACT/DVE computes exp.
                pending_ctx = []
                CTX_LAG = 2

                def do_ctx(et, qc, kt, ex_a, ex_b):
                    hA, hB = 2 * et, 2 * et + 1
                    key = (et, qc)
                    if key not in p2state:
                        cpA = cpp.tile([128, 260], F32, tag="cp",
                                       name=f"cpA_{et}_{qc}")
                        cpB = cpp.tile([128, 260], F32, tag="cp",
                                       name=f"cpB_{et}_{qc}")
                        p2state[key] = (cpA, cpB)
                    cpA, cpB = p2state[key]
                    # PSUM start_tensor_calc zeroing is bank-wide: only the
                    # first matmul touching each cp bank may start, or it
                    # wipes the other column-groups' accumulation.
                    for cpx, ex16, hl in ((cpA, ex_a, hA), (cpB, ex_b, hB)):
                        for j in range(4):
                            nc.tensor.matmul(
                                cpx[:, j * 65:j * 65 + 65],
                                ex16[:, j * 128:j * 128 + 128],
                                vp_t[kt][:, hl * 65:hl * 65 + 65],
                                start=(kt == 0 and j == 0),
                                stop=(kt == NKT - 1 and j == 3),
                                skip_group_check=True)

                def drain_ctx(n):
                    while len(pending_ctx) > n:
                        do_ctx(*pending_ctx.pop(0))

                def p2_kloop(et, qc, kts):
                    for kt in kts:
                        ktt = kt_t[et][kt // 4]
                        qtt = qt_t[et][qc]
                        ko = (kt % 4) * 128
                        exs = [None, None]
                        for hl2 in (0, 1):
                            sps = ssp.tile([128, QW], F32, tag="ss",
                                           name=f"sps_{et}_{qc}_{kt}_{hl2}")
                            nc.tensor.matmul(
                                sps[:],
                                ktt[hl2 * 64:hl2 * 64 + 64, ko:ko + 128],
                                qtt[hl2 * 64:hl2 * 64 + 64, :],
                                start=True, stop=True)
                            exa = exa_pool.tile([128, QW], F16, tag="exa",
                                                name=f"exa_{et}_{qc}_{kt}_{hl2}")
                            if with_sch and (kt + hl2) % 2 == 0:
                                nc.vector.tensor_scalar(
                                    exa[:].bitcast(I16),
                                    sps[:], SCH_A * 0.125,
                                    smask_sb[:, kt:kt + 1], Mult, Add)
                            else:
                                nc.scalar.activation(
                                    exa[:], sps[:], Exp,
                                    bias=mask_sb[:, kt:kt + 1], scale=0.125)
                            exs[hl2] = exa[:]
                        pending_ctx.append((et, qc, kt, exs[0], exs[1]))
                        drain_ctx(CTX_LAG)

                def p2_epilogue(et, qc):
                    # make sure all of this unit's ctx matmuls are emitted
                    while any(p[0] == et and p[1] == qc for p in pending_ctx):
                        do_ctx(*pending_ctx.pop(0))
                    cpA, cpB = p2state.pop((et, qc))
                    og = ob_pool.tile([128, 512], F32, tag="ob",
                                      name=f"ob_{et}_{qc}")
                    ogv = og.rearrange("p (j c) -> p j c", c=128)
                    for hl2, cpx in ((0, cpA), (1, cpB)):
                        cpv = cpx.rearrange("p (j c) -> p j c", c=65)
                        rc = rc_pool.tile([128, 4], F32, tag="rc",
                                          name=f"rc_{et}_{qc}_{hl2}")
                        nc.vector.reciprocal(rc[:], cpv[:, :, 64:65])
                        # per-partition scale on ACT (Copy activation) --
                        # keeps the DVE free for the Schraudolph exps
                        for j in range(4):
                            nc.scalar.mul(
                                ogv[:, j, hl2 * 64:hl2 * 64 + 64],
                                cpv[:, j, 0:64], rc[:, j:j + 1])
                    od = out[qc * QW:(qc + 1) * QW, et * 128:(et + 1) * 128]
                    od = od.rearrange("(j p) c -> p j c", p=128)
                    nc.sync.dma_start(od, ogv)

                def proj_qk(w_t, b_sb, dst, xts, s0):
                    sc = s0 // 512
                    for et in range(NET):
                        p = ssp.tile([128, 512], F32, tag="ss",
                                     name=f"pj_{s0}_{et}")
                        for h in range(NHT):
                            nc.tensor.matmul(
                                p[:],
                                w_t[h][:, et * 128:et * 128 + 128],
                                xts[h][:],
                                start=(h == 0), stop=(h == NHT - 1))
                        nc.vector.tensor_scalar_add(
                            dst[et][sc][:], p[:], b_sb[:, et:et + 1])

                all_xts = {}
                for sc in range(4):  # s-chunks of 512
                    s0 = sc * 512
                    if sc == 0:
                        xts = xts0
                    else:
                        xts = []
                        for h in range(NHT):
                            t = xtp.tile([128, 512], F16, tag="xt",
                                         name=f"xt{sc}_{h}")
                            nc.sync.dma_start(t[:], xt[h * 128:(h + 1) * 128,
                                                       s0:s0 + 512])
                            xts.append(t)
                    all_xts[sc] = xts

                    proj_qk(wk_t, bk_sb, kt_t, xts, s0)

                    # V s-tiles: out [s 128, e 512] (+ bias via K=1 matmul)
                    for st in range(4):
                        gst = sc * 4 + st
                        p = ssp.tile([128, 512], F32, tag="ss")
                        for h in range(NHT):
                            nc.tensor.matmul(
                                p[:],
                                xts[h][:, st * 128:st * 128 + 128],
                                wv_t[h][:],
                                start=(h == 0),
                                stop=(not with_vbias and h == NHT - 1))
                        if with_vbias:
                            nc.tensor.matmul(p[:], ones_row[:], bv_sb[:],
                                             start=False, stop=True)
                        dstv = vp_t[gst].rearrange("p (h c) -> p h c", c=65)
                        nc.vector.tensor_copy(
                            dstv[:, :, 0:64],
                            p.rearrange("p (h c) -> p h c", c=64))

                    # Q for early q-chunks only; chunks 2,3 are deferred into
                    # phase 2 where PE micro-gaps absorb them
                    if sc < 2:
                        proj_qk(wq_t, bq_sb, qt_t, xts, s0)

                    # head-pair 0 / q-chunk 0: this chunk's k-tiles are ready
                    # now -> feeds ACT during phase 1
                    p2_kloop(0, 0, range(4 * sc, 4 * sc + 4))

                # ---------------- Phase 2: attention (head pairs) -------
                # epilogues trail one unit behind so each unit's last ctx
                # matmuls pipeline into the next unit's scores.
                deferred_q = [2, 3]
                prev_unit = (0, 0)
                for et in range(NET):
                    for qc in range(S // QW):
                        if et == 0 and deferred_q and qc == deferred_q[0] - 2:
                            sc = deferred_q.pop(0)
                            proj_qk(wq_t, bq_sb, qt_t, all_xts[sc],
                                    sc * 512)
                        if (et, qc) == (0, 0):
                            continue
                        p2_kloop(et, qc, range(NKT))
                        p2_epilogue(*prev_unit)
                        prev_unit = (et, qc)
                drain_ctx(0)
                p2_epilogue(*prev_unit)

    nc.compile()
    return nc


def build_in_maps(inputs, with_vbias=None, with_sch=None):
    attention_mask = np.asarray(inputs["attention_mask"], dtype=np.float32)
    if with_vbias is None:
        with_vbias = bool(np.any(np.asarray(inputs["bv"], np.float32)))
    if with_sch is None:
        with_sch = bool(np.abs(attention_mask).max() < 40.0)
    suffix = f"{KERNEL_VERSION}{'b' if with_vbias else ''}{'s' if with_sch else ''}"
    hidden_states = np.asarray(inputs["hidden_states"], dtype=np.float32)
    Wq, bq = np.asarray(inputs["Wq"], np.float32), np.asarray(inputs["bq"], np.float32)
    Wk, bk = np.asarray(inputs["Wk"], np.float32), np.asarray(inputs["bk"], np.float32)
    Wv, bv = np.asarray(inputs["Wv"], np.float32), np.asarray(inputs["bv"], np.float32)

    xts = [np.ascontiguousarray(hidden_states[b].T).astype(np.float16)
           for b in range(B)]
    masks = [np.ascontiguousarray(attention_mask[b, 0, 0].reshape(NKT, 128).T)
             for b in range(B)]
    smasks = [(SCH_B + SCH_A * m).astype(np.float32) for m in masks]
    wg = []
    for g in range(2):
        rows = slice(g * E, (g + 1) * E)
        wg.append({
            "wqt": np.ascontiguousarray(Wq[rows].T).astype(np.float16),
            "wkt": np.ascontiguousarray(Wk[rows].T).astype(np.float16),
            "wvt": np.ascontiguousarray(Wv[rows].T).astype(np.float16),
            "bq2": np.ascontiguousarray(bq[rows].reshape(NET, 128).T),
            "bk2": np.ascontiguousarray(bk[rows].reshape(NET, 128).T),
            "bv2": np.ascontiguousarray(bv[rows].reshape(1, E)).astype(np.float16),
        })
    in_maps = []
    for c in range(NCORES):
        b, g = c // 2, c % 2
        in_maps.append({
            "xt": xts[b],
            f"mask2_{suffix}": masks[b],
            "smask2": smasks[b],
            **wg[g],
        })
    return in_maps


def kernel(hidden_states, attention_mask, Wq, bq, Wk, bk, Wv, bv):
    with_vbias = bool(np.any(np.asarray(bv, np.float32)))
    with_sch = bool(np.abs(np.asarray(attention_mask, np.float32)).max() < 40.0)
    ckey = ("nc", with_vbias, with_sch)
    if ckey not in _CACHE:
        _CACHE[ckey] = build_kernel(with_vbias, with_sch)
    nc = _CACHE[ckey]

    in_maps = build_in_maps(dict(
        hidden_states=hidden_states, attention_mask=attention_mask,
        Wq=Wq, bq=bq, Wk=Wk, bk=bk, Wv=Wv, bv=bv),
        with_vbias=with_vbias, with_sch=with_sch)

    trace = bool(int(os.environ.get("BASS_KERNEL_TRACE", "0")))
    res = run_bass_kernel_spmd(nc, in_maps, core_ids=list(range(NCORES)),
                               trace=trace)
    LAST_PROFILE["exec_time_ns"] = res.exec_time_ns
    LAST_PROFILE["mean_exec_time_ns"] = res.mean_exec_time_ns
    if res.instructions_and_trace is not None:
        LAST_PROFILE["trace_path"] = res.instructions_and_trace[1]

    full = np.empty((B, S, H), dtype=np.float32)
    for c in range(NCORES):
        b, g = c // 2, c % 2
        full[b][:, g * E:(g + 1) * E] = res.results[c]["out"]
    return full


# revision 34
# speedup vs baseline: 1.3944x; 1.1004x over previous
"""BERT self-attention on 8 TRN2 NeuronCores.

Problem: hidden_states [4, 2048, 1024], 16 heads x 64 dim, fp32.
Sharding: core c handles batch b = c//2 and head-group g = c%2
(8 heads = 512 embedding columns per core). Full inputs in, full
output out; slicing/transposition of inputs happens host-side here.

v14 design (per-core):
  All matmul operands fp16 (error budget allows: rel err ~8e-3 vs
  the 2e-2 gate); PSUM accumulation fp32.
  Phase 1: Q^T/K^T [e,s] and V [s,e] projections from X^T [h,s],
           weights pre-transposed host-side, all fp16. Q/K biases via
           DVE add on the PSUM->SBUF copy; V bias (when nonzero) via a
           K=1 ones-row matmul. V' gets a ones column per head
           (softmax denominator trick) via a tile-wide memset to 1.0
           before the 64-col blocks are copied in.
  Phase 2: per (head-pair et, q-chunk qc of 512):
           S^T[k,q] = K^T.T @ Q^T (K=64 contraction, N=512 moving),
           expS = exp(S*0.125 + mask_k): most k-tiles on ACT (true
           exp -> fp16), a tunable subset on DVE via the Schraudolph
           trick (bits = A*arg + B as int16, bitcast to fp16; ~3% rel
           err, consistent numerator/denominator so it mostly cancels).
           ctx: queries in the PE partition dim: cp[128q, 4j x 65] +=
           ex[k, q-tile].T @ V'[k, 65] -- N=65 per matmul instead of
           512, 2x fewer PE cycles than the [65, q] orientation, and
           the [q, 65] output needs no PE transpose in the epilogue.
           Epilogue: DVE reciprocal of col 64, per-partition scale of
           cols 0:64, one [128, 4, 128] DMA per (et, qc) to out.
"""

import os
import numpy as np

import concourse.bass as bass
import concourse.tile as tile
from concourse import bacc, mybir
from concourse.bass_utils import run_bass_kernel_spmd

F32 = mybir.dt.float32
F16 = mybir.dt.float16
I16 = mybir.dt.int16

B, S, H = 4, 2048, 1024
NH, HD = 16, 64
NCORES = 8
E = 512          # embedding columns per core (8 heads)
NHL = 8          # heads per core
NKT = S // 128   # 16 k-tiles
NET = E // 128   # 4 e-tiles (head pairs)
NHT = H // 128   # 8 h-tiles
QW = 512         # per-head q-chunk width

# Schraudolph fast-exp constants (fp16 bits = round(A*arg + B16))
SCH_A = 1024.0 / float(np.log(2.0))
SCH_B = 1024.0 * 15 - 44.25
# Per (k-tile, head) the scores land in their own 1-bank PSUM tile and
# the exp runs as one full-tile instruction: one head on ACT (true exp),
# the other on DVE (Schraudolph bits trick), swapping per k-tile parity.
# Both engines run concurrently, halving the exp latency in the
# scores->exp->ctx chain; every softmax row is 50/50 exact/approximate.

_CACHE = {}

KERNEL_VERSION = "v14"  # bump to bust the neuron compile cache on kernel changes

LAST_PROFILE = {}


def build_kernel(with_vbias=True, with_sch=True):
    nc = bacc.Bacc("TRN2", target_bir_lowering=False, debug=False,
                   num_devices=NCORES)

    xt = nc.dram_tensor("xt", [H, S], F16, kind="ExternalInput").ap()
    wqt = nc.dram_tensor("wqt", [H, E], F16, kind="ExternalInput").ap()
    wkt = nc.dram_tensor("wkt", [H, E], F16, kind="ExternalInput").ap()
    wvt = nc.dram_tensor("wvt", [H, E], F16, kind="ExternalInput").ap()
    bq2 = nc.dram_tensor("bq2", [128, NET], F32, kind="ExternalInput").ap()
    bk2 = nc.dram_tensor("bk2", [128, NET], F32, kind="ExternalInput").ap()
    bv2 = nc.dram_tensor("bv2", [1, E], F16, kind="ExternalInput").ap()
    suffix = f"{KERNEL_VERSION}{'b' if with_vbias else ''}{'s' if with_sch else ''}"
    mask2 = nc.dram_tensor(f"mask2_{suffix}", [128, NKT], F32,
                           kind="ExternalInput").ap()
    smask2 = nc.dram_tensor("smask2", [128, NKT], F32,
                            kind="ExternalInput").ap()
    out = nc.dram_tensor("out", [S, E], F32, kind="ExternalOutput").ap()

    Exp = mybir.ActivationFunctionType.Exp
    Mult, Add = mybir.AluOpType.mult, mybir.AluOpType.add

    with tile.TileContext(nc) as tc:
        with (
            tc.tile_pool(name="persist", bufs=1) as persist,
            tc.tile_pool(name="small", bufs=1) as small,
        ):
            # persistent SBUF tensors, split per chunk so each has a
            # single producer -> exact dependencies, phases can overlap
            qt_t = [[persist.tile([128, 512], F16, name=f"qt_{et}_{sc}")
                     for sc in range(4)] for et in range(NET)]
            kt_t = [[persist.tile([128, 512], F16, name=f"kt_{et}_{sc}")
                     for sc in range(4)] for et in range(NET)]
            vp_t = [persist.tile([128, NHL * 65], F16, name=f"vp_{gst}")
                    for gst in range(NKT)]

            # small input tiles: DMAs are emitted after the big X/weight
            # loads (single HWDGE queue; each descriptor-gen is ~625ns)
            mask_sb = small.tile([128, NKT], F32)
            smask_sb = small.tile([128, NKT], F32)
            bq_sb = small.tile([128, NET], F32)
            bk_sb = small.tile([128, NET], F32)
            bv_sb = small.tile([1, E], F16)
            ones_row = small.tile([1, 128], F16)
            nc.vector.memset(ones_row[:], 1.0)

            # ones columns of V' (denominator trick): memset whole tile
            # to 1.0; the V-projection copies later overwrite cols 0:64
            # of each head's 65-block, leaving col 64 = 1.0. On the idle
            # Pool engine to keep early DVE cycles free.
            for gst in range(NKT):
                nc.gpsimd.memset(vp_t[gst][:], 1.0)

            # ---- unified pools (phases overlap at runtime) ----
            # PSUM banks: ss(proj+scores) 4 slots x 1 bank ([128,512] f32),
            #             cp 4 x 1 bank ([128, 260] f32) -> 8 total
            with (
                tc.tile_pool(name="xtp", bufs=3) as xtp,
                tc.tile_pool(name="wp", bufs=1) as wp,
                tc.tile_pool(name="ssp", bufs=4, space="PSUM") as ssp,
                tc.tile_pool(name="cpp", bufs=4, space="PSUM") as cpp,
                tc.tile_pool(name="exa", bufs=10) as exa_pool,
                tc.tile_pool(name="obp", bufs=3) as ob_pool,
                tc.tile_pool(name="rcp", bufs=6) as rc_pool,
            ):
                # ---------------- Phase 1: projections ----------------
                # batched loads: each HWDGE descriptor-gen costs ~625ns
                # regardless of size, so X / weights load as [128, 4h, 512]
                # chunks (2 DMAs per tensor), interleaved so the first
                # K-projection matmuls can start after two DMAs.
                def load_x(sc, chunks=(0, 4)):
                    t = xtp.tile([128, NHT, 512], F16, tag="xt",
                                 name=f"xt{sc}")
                    s0 = sc * 512
                    for i, hh in enumerate(chunks):
                        nh = (chunks[i + 1] if i + 1 < len(chunks) else NHT) - hh
                        src = xt[hh * 128:(hh + nh) * 128, s0:s0 + 512]
                        nc.sync.dma_start(
                            t[:, hh:hh + nh, :],
                            src.rearrange("(h p) s -> p h s", p=128))
                    return t

                wkb = wp.tile([128, NHT, E], F16, name="wkb")
                wvb = wp.tile([128, NHT, E], F16, name="wvb")
                wqb = wp.tile([128, NHT, E], F16, name="wqb")

                def load_w(wb, wsrc, hh, nh=4):
                    src = wsrc[hh * 128:(hh + nh) * 128, :]
                    nc.sync.dma_start(
                        wb[:, hh:hh + nh, :],
                        src.rearrange("(h p) e -> p h e", p=128))

                # fine-grained first chunks so the first K-proj matmuls can
                # begin while the rest of X/W streams in
                xbig0 = xtp.tile([128, NHT, 512], F16, tag="xt", name="xt0")
                nc.sync.dma_start(
                    xbig0[:, 0:2, :],
                    xt[0:256, 0:512].rearrange("(h p) s -> p h s", p=128))
                load_w(wkb, wkt, 0, 2)
                nc.sync.dma_start(
                    xbig0[:, 2:4, :],
                    xt[256:512, 0:512].rearrange("(h p) s -> p h s", p=128))
                load_w(wkb, wkt, 2, 2)
                nc.sync.dma_start(
                    xbig0[:, 4:8, :],
                    xt[512:1024, 0:512].rearrange("(h p) s -> p h s", p=128))
                load_w(wkb, wkt, 4, 4)
                for hh in (0, 4):
                    load_w(wvb, wvt, hh)
                nc.sync.dma_start(mask_sb[:], mask2)
                nc.sync.dma_start(smask_sb[:], smask2)
                nc.sync.dma_start(bk_sb[:], bk2)
                for hh in (0, 4):
                    load_w(wqb, wqt, hh)
                nc.sync.dma_start(bq_sb[:], bq2)
                nc.sync.dma_start(bv_sb[:], bv2)

                p2state = {}
                # software pipeline: ctx(kt) is emitted ~CTX_LAG k-steps
                # after its scores/exp, so the in-order PE always has the
                # next scores ready while ACT/DVE computes exp.
                pending_ctx = []
                CTX_LAG = 2

                def do_ctx(et, qc, kt, ex_a, ex_b):
                    hA, hB = 2 * et, 2 * et + 1
                    key = (et, qc)
                    if key not in p2state:
                        cpA = cpp.tile([128, 260], F32, tag="cp",
                                       name=f"cpA_{et}_{qc}")
                        cpB = cpp.tile([128, 260], F32, tag="cp",
                                       name=f"cpB_{et}_{qc}")
                        p2state[key] = (cpA, cpB)
                    cpA, cpB = p2state[key]
                    # PSUM start_tensor_calc zeroing is bank-wide: only the
                    # first matmul touching each cp bank may start, or it
                    # wipes the other column-groups' accumulation.
                    for cpx, ex16, hl in ((cpA, ex_a, hA), (cpB, ex_b, hB)):
                        for j in range(4):
                            nc.tensor.matmul(
                                cpx[:, j * 65:j * 65 + 65],
                                ex16[:, j * 128:j * 128 + 128],
                                vp_t[kt][:, hl * 65:hl * 65 + 65],
                                start=(kt == 0 and j == 0),
                                stop=(kt == NKT - 1 and j == 3),
                                skip_group_check=True)

                def drain_ctx(n):
                    while len(pending_ctx) > n:
                        do_ctx(*pending_ctx.pop(0))

                def p2_kloop(et, qc, kts):
                    for kt in kts:
                        ktt = kt_t[et][kt // 4]
                        qtt = qt_t[et][qc]
                        ko = (kt % 4) * 128
                        exs = [None, None]
                        for hl2 in (0, 1):
                            sps = ssp.tile([128, QW], F32, tag="ss",
                                           name=f"sps_{et}_{qc}_{kt}_{hl2}")
                            nc.tensor.matmul(
                                sps[:],
                                ktt[hl2 * 64:hl2 * 64 + 64, ko:ko + 128],
                                qtt[hl2 * 64:hl2 * 64 + 64, :],
                                start=True, stop=True)
                            exa = exa_pool.tile([128, QW], F16, tag="exa",
                                                name=f"exa_{et}_{qc}_{kt}_{hl2}")
                            if with_sch and (kt + hl2) % 2 == 0:
                                nc.vector.tensor_scalar(
                                    exa[:].bitcast(I16),
                                    sps[:], SCH_A * 0.125,
                                    smask_sb[:, kt:kt + 1], Mult, Add)
                            else:
                                nc.scalar.activation(
                                    exa[:], sps[:], Exp,
                                    bias=mask_sb[:, kt:kt + 1], scale=0.125)
                            exs[hl2] = exa[:]
                        pending_ctx.append((et, qc, kt, exs[0], exs[1]))
                        drain_ctx(CTX_LAG)

                def p2_epilogue(et, qc):
                    # make sure all of this unit's ctx matmuls are emitted
                    while any(p[0] == et and p[1] == qc for p in pending_ctx):
                        do_ctx(*pending_ctx.pop(0))
                    cpA, cpB = p2state.pop((et, qc))
                    og = ob_pool.tile([128, 512], F32, tag="ob",
                                      name=f"ob_{et}_{qc}")
                    ogv = og.rearrange("p (j c) -> p j c", c=128)
                    for hl2, cpx in ((0, cpA), (1, cpB)):
                        cpv = cpx.rearrange("p (j c) -> p j c", c=65)
                        rc = rc_pool.tile([128, 4], F32, tag="rc",
                                          name=f"rc_{et}_{qc}_{hl2}")
                        nc.vector.reciprocal(rc[:], cpv[:, :, 64:65])
                        # per-partition scale on ACT (Copy activation) --
                        # keeps the DVE free for the Schraudolph exps
                        for j in range(4):
                            nc.scalar.mul(
                                ogv[:, j, hl2 * 64:hl2 * 64 + 64],
                                cpv[:, j, 0:64], rc[:, j:j + 1])
                    od = out[qc * QW:(qc + 1) * QW, et * 128:(et + 1) * 128]
                    od = od.rearrange("(j p) c -> p j c", p=128)
                    nc.sync.dma_start(od, ogv)

                def proj_piece(wb, b_sb, dst, xb, s0, et):
                    sc = s0 // 512
                    p = ssp.tile([128, 512], F32, tag="ss",
                                 name=f"pj_{s0}_{et}")
                    for h in range(NHT):
                        nc.tensor.matmul(
                            p[:],
                            wb[:, h, et * 128:et * 128 + 128],
                            xb[:, h, :],
                            start=(h == 0), stop=(h == NHT - 1))
                    nc.vector.tensor_scalar_add(
                        dst[et][sc][:], p[:], b_sb[:, et:et + 1])

                def proj_qk(wb, b_sb, dst, xb, s0):
                    for et in range(NET):
                        proj_piece(wb, b_sb, dst, xb, s0, et)

                all_xts = {}
                for sc in range(4):  # s-chunks of 512
                    s0 = sc * 512
                    xb = xbig0 if sc == 0 else load_x(sc)
                    all_xts[sc] = xb

                    proj_qk(wkb, bk_sb, kt_t, xb, s0)

                    # V s-tiles: out [s 128, e 512] (+ bias via K=1 matmul)
                    for st in range(4):
                        gst = sc * 4 + st
                        p = ssp.tile([128, 512], F32, tag="ss")
                        for h in range(NHT):
                            nc.tensor.matmul(
                                p[:],
                                xb[:, h, st * 128:st * 128 + 128],
                                wvb[:, h, :],
                                start=(h == 0),
                                stop=(not with_vbias and h == NHT - 1))
                        if with_vbias:
                            nc.tensor.matmul(p[:], ones_row[:], bv_sb[:],
                                             start=False, stop=True)
                        dstv = vp_t[gst].rearrange("p (h c) -> p h c", c=65)
                        nc.vector.tensor_copy(
                            dstv[:, :, 0:64],
                            p.rearrange("p (h c) -> p h c", c=64))

                    # Q for early q-chunks only; chunks 2,3 are deferred into
                    # phase 2 where PE micro-gaps absorb them
                    if sc < 2:
                        proj_qk(wqb, bq_sb, qt_t, xb, s0)

                    # head-pair 0 / q-chunk 0: this chunk's k-tiles are ready
                    # now -> feeds ACT during phase 1
                    p2_kloop(0, 0, range(4 * sc, 4 * sc + 4))

                # ---------------- Phase 2: attention (head pairs) -------
                # epilogues trail one unit behind so each unit's last ctx
                # matmuls pipeline into the next unit's scores.
                # deferred Q projections: one (sc, et) piece per unit,
                # spread across the first 8 units so each 1.7us PE burst
                # hides in a single unit's exp slack
                deferred_q = [(sc, et) for et in range(NET) for sc in (2, 3)]
                prev_unit = (0, 0)
                ui = 0
                for et in range(NET):
                    for qc in range(S // QW):
                        if (et, qc) == (0, 0):
                            continue
                        ui += 1
                        if deferred_q and ui >= 1:
                            dsc, det = deferred_q.pop(0)
                            # must be projected before unit (0, qc=dsc) needs
                            # qt_t[*][dsc]
                            proj_piece(wqb, bq_sb, qt_t, all_xts[dsc],
                                       dsc * 512, det)
                        p2_kloop(et, qc, range(NKT))
                        p2_epilogue(*prev_unit)
                        prev_unit = (et, qc)
                drain_ctx(0)
                p2_epilogue(*prev_unit)

    nc.compile()
    return nc


def build_in_maps(inputs, with_vbias=None, with_sch=None):
    attention_mask = np.asarray(inputs["attention_mask"], dtype=np.float32)
    if with_vbias is None:
        with_vbias = bool(np.any(np.asarray(inputs["bv"], np.float32)))
    if with_sch is None:
        with_sch = bool(np.abs(attention_mask).max() < 40.0)
    suffix = f"{KERNEL_VERSION}{'b' if with_vbias else ''}{'s' if with_sch else ''}"
    hidden_states = np.asarray(inputs["hidden_states"], dtype=np.float32)
    Wq, bq = np.asarray(inputs["Wq"], np.float32), np.asarray(inputs["bq"], np.float32)
    Wk, bk = np.asarray(inputs["Wk"], np.float32), np.asarray(inputs["bk"], np.float32)
    Wv, bv = np.asarray(inputs["Wv"], np.float32), np.asarray(inputs["bv"], np.float32)

    xts = [np.ascontiguousarray(hidden_states[b].T).astype(np.float16)
           for b in range(B)]
    masks = [np.ascontiguousarray(attention_mask[b, 0, 0].reshape(NKT, 128).T)
             for b in range(B)]
    smasks = [(SCH_B + SCH_A * m).astype(np.float32) for m in masks]
    wg = []
    for g in range(2):
        rows = slice(g * E, (g + 1) * E)
        wg.append({
            "wqt": np.ascontiguousarray(Wq[rows].T).astype(np.float16),
            "wkt": np.ascontiguousarray(Wk[rows].T).astype(np.float16),
            "wvt": np.ascontiguousarray(Wv[rows].T).astype(np.float16),
            "bq2": np.ascontiguousarray(bq[rows].reshape(NET, 128).T),
            "bk2": np.ascontiguousarray(bk[rows].reshape(NET, 128).T),
            "bv2": np.ascontiguousarray(bv[rows].reshape(1, E)).astype(np.float16),
        })
    in_maps = []
    for c in range(NCORES):
        b, g = c // 2, c % 2
        in_maps.append({
            "xt": xts[b],
            f"mask2_{suffix}": masks[b],
            "smask2": smasks[b],
            **wg[g],
        })
    return in_maps


def kernel(hidden_states, attention_mask, Wq, bq, Wk, bk, Wv, bv):
    with_vbias = bool(np.any(np.asarray(bv, np.float32)))
    with_sch = bool(np.abs(np.asarray(attention_mask, np.float32)).max() < 40.0)
    ckey = ("nc", with_vbias, with_sch)
    if ckey not in _CACHE:
        _CACHE[ckey] = build_kernel(with_vbias, with_sch)
    nc = _CACHE[ckey]

    in_maps = build_in_maps(dict(
        hidden_states=hidden_states, attention_mask=attention_mask,
        Wq=Wq, bq=bq, Wk=Wk, bk=bk, Wv=Wv, bv=bv),
        with_vbias=with_vbias, with_sch=with_sch)

    trace = bool(int(os.environ.get("BASS_KERNEL_TRACE", "0")))
    res = run_bass_kernel_spmd(nc, in_maps, core_ids=list(range(NCORES)),
                               trace=trace)
    LAST_PROFILE["exec_time_ns"] = res.exec_time_ns
    LAST_PROFILE["mean_exec_time_ns"] = res.mean_exec_time_ns
    if res.instructions_and_trace is not None:
        LAST_PROFILE["trace_path"] = res.instructions_and_trace[1]

    full = np.empty((B, S, H), dtype=np.float32)
    for c in range(NCORES):
        b, g = c // 2, c % 2
        full[b][:, g * E:(g + 1) * E] = res.results[c]["out"]
    return full


# revision 60
# speedup vs baseline: 1.4505x; 1.0402x over previous
"""BERT self-attention on 8 TRN2 NeuronCores.

Problem: hidden_states [4, 2048, 1024], 16 heads x 64 dim, fp32.
Sharding: core c handles batch b = c//2 and head-group g = c%2
(8 heads = 512 embedding columns per core). Full inputs in, full
output out; slicing/transposition of inputs happens host-side here.

v14 design (per-core), HW exec ~273us (cost-model timeline) vs the
396us f32r baseline. The cost model charges a matmul only for its
moving-operand columns (1 cycle/row for fp16 at any size), so:

  All matmul operands fp16 (rel err 9.7e-3 vs the 2e-2 gate); PSUM
  accumulation fp32.
  Phase 1: Q^T/K^T [e,s] and V [s,e] projections from X^T [h,s],
           weights pre-transposed host-side, all fp16, loaded as
           [128, 4h, 512] batched DMAs (each HWDGE descriptor-gen is
           ~625ns, so few big DMAs beat many small ones; the first
           X/Wk chunks are h-pair sized so the first matmuls start
           ~4us in). Q/K biases via DVE add on the PSUM->SBUF copy;
           V bias (when nonzero) via a K=1 ones-row matmul. V' gets a
           ones column per head (softmax denominator trick) via a
           Pool-engine memset to 1.0 before the V columns land.
  Phase 2: per (head-pair et, q-chunk qc of 512), per k-tile:
           each head's scores S^T[k,q] go to their own 1-bank PSUM
           tile (4 rotating slots); exp(S*0.125 + mask_k) runs as one
           full-tile instruction per head, one head on ACT (true exp
           -> fp16) and the other on DVE (Schraudolph: fp16 bits =
           A*arg + B as int16, bitcast to fp16; ~3% rel err that
           mostly cancels between numerator and denominator), parity
           swapping per k-tile so both engines run concurrently and
           every softmax row is 50/50 exact/approx.
           ctx: queries in the PE partition dim: cp[128q, 4j x 65] +=
           ex[k, q-tile].T @ V'[k, 65] -- the fp16 moving operand is
           only 65 wide, 2x fewer PE cycles than the [65, q]
           orientation, and no PE transpose in the epilogue. ctx
           emission trails scores/exp by CTX_LAG k-tiles (software
           pipelining: the in-order PE never stalls on exp latency).
           PSUM start_tensor_calc zeroing is bank-wide, so only the
           first matmul touching a cp bank may set start=True.
           Units (0,0) and (1,0) run during phase 1 (their exp load
           hides under projection PE time); 13 of the 16 Q-projection
           (sc, et) pieces are deferred, one per attention unit in
           deadline order, as PE filler for the exp-bound units; the
           remaining X s-chunks are prefetched behind the weights.
           Epilogue (trails one unit): DVE reciprocal of col 64,
           per-partition scales split across ACT (Copy activation
           with scale AP) and DVE, one [128, 4, 128] DMA to out.
"""

import os
import numpy as np

import concourse.bass as bass
import concourse.tile as tile
from concourse import bacc, mybir
from concourse.bass_utils import run_bass_kernel_spmd

F32 = mybir.dt.float32
F16 = mybir.dt.float16
I16 = mybir.dt.int16

B, S, H = 4, 2048, 1024
NH, HD = 16, 64
NCORES = 8
E = 512          # embedding columns per core (8 heads)
NHL = 8          # heads per core
NKT = S // 128   # 16 k-tiles
NET = E // 128   # 4 e-tiles (head pairs)
NHT = H // 128   # 8 h-tiles
QW = 512         # per-head q-chunk width

# Schraudolph fast-exp constants (fp16 bits = round(A*arg + B16))
SCH_A = 1024.0 / float(np.log(2.0))
SCH_B = 1024.0 * 15 - 44.25
# Per (k-tile, head) the scores land in their own 1-bank PSUM tile and
# the exp runs as one full-tile instruction: one head on ACT (true exp),
# the other on DVE (Schraudolph bits trick), swapping per k-tile parity.
# Both engines run concurrently, halving the exp latency in the
# scores->exp->ctx chain; every softmax row is 50/50 exact/approximate.

_CACHE = {}

KERNEL_VERSION = "v14"  # bump to bust the neuron compile cache on kernel changes

LAST_PROFILE = {}


def build_kernel(with_vbias=True, with_sch=True):
    nc = bacc.Bacc("TRN2", target_bir_lowering=False, debug=False,
                   num_devices=NCORES)

    xt = nc.dram_tensor("xt", [H, S], F16, kind="ExternalInput").ap()
    wqt = nc.dram_tensor("wqt", [H, E], F16, kind="ExternalInput").ap()
    wkt = nc.dram_tensor("wkt", [H, E], F16, kind="ExternalInput").ap()
    wvt = nc.dram_tensor("wvt", [H, E], F16, kind="ExternalInput").ap()
    bq2 = nc.dram_tensor("bq2", [128, NET], F32, kind="ExternalInput").ap()
    bk2 = nc.dram_tensor("bk2", [128, NET], F32, kind="ExternalInput").ap()
    bv2 = nc.dram_tensor("bv2", [1, E], F16, kind="ExternalInput").ap()
    suffix = f"{KERNEL_VERSION}{'b' if with_vbias else ''}{'s' if with_sch else ''}"
    mask2 = nc.dram_tensor(f"mask2_{suffix}", [128, NKT], F32,
                           kind="ExternalInput").ap()
    smask2 = nc.dram_tensor("smask2", [128, NKT], F32,
                            kind="ExternalInput").ap()
    out = nc.dram_tensor("out", [S, E], F32, kind="ExternalOutput").ap()

    Exp = mybir.ActivationFunctionType.Exp
    Mult, Add = mybir.AluOpType.mult, mybir.AluOpType.add

    with tile.TileContext(nc) as tc:
        with (
            tc.tile_pool(name="persist", bufs=1) as persist,
            tc.tile_pool(name="small", bufs=1) as small,
        ):
            # persistent SBUF tensors, split per chunk so each has a
            # single producer -> exact dependencies, phases can overlap
            qt_t = [[persist.tile([128, 512], F16, name=f"qt_{et}_{sc}")
                     for sc in range(4)] for et in range(NET)]
            kt_t = [[persist.tile([128, 512], F16, name=f"kt_{et}_{sc}")
                     for sc in range(4)] for et in range(NET)]
            vp_t = [persist.tile([128, NHL * 65], F16, name=f"vp_{gst}")
                    for gst in range(NKT)]

            # small input tiles: DMAs are emitted after the big X/weight
            # loads (single HWDGE queue; each descriptor-gen is ~625ns)
            mask_sb = small.tile([128, NKT], F32)
            smask_sb = small.tile([128, NKT], F32)
            bq_sb = small.tile([128, NET], F32)
            bk_sb = small.tile([128, NET], F32)
            bv_sb = small.tile([1, E], F16)
            ones_row = small.tile([1, 128], F16)
            nc.vector.memset(ones_row[:], 1.0)

            # ones columns of V' (denominator trick): memset whole tile
            # to 1.0; the V-projection copies later overwrite cols 0:64
            # of each head's 65-block, leaving col 64 = 1.0. On the idle
            # Pool engine to keep early DVE cycles free.
            for gst in range(NKT):
                nc.gpsimd.memset(vp_t[gst][:], 1.0)

            # ---- unified pools (phases overlap at runtime) ----
            # PSUM banks: ss(proj+scores) 4 slots x 1 bank ([128,512] f32),
            #             cp 4 x 1 bank ([128, 260] f32) -> 8 total
            with (
                tc.tile_pool(name="xtp", bufs=4) as xtp,
                tc.tile_pool(name="wp", bufs=1) as wp,
                tc.tile_pool(name="ssp", bufs=4, space="PSUM") as ssp,
                tc.tile_pool(name="cpp", bufs=4, space="PSUM") as cpp,
                tc.tile_pool(name="exa", bufs=12) as exa_pool,
                tc.tile_pool(name="obp", bufs=4) as ob_pool,
                tc.tile_pool(name="rcp", bufs=8) as rc_pool,
            ):
                # ---------------- Phase 1: projections ----------------
                # batched loads: each HWDGE descriptor-gen costs ~625ns
                # regardless of size, so X / weights load as [128, 4h, 512]
                # chunks (2 DMAs per tensor), interleaved so the first
                # K-projection matmuls can start after two DMAs.
                def load_x(sc, chunks=(0, 4)):
                    t = xtp.tile([128, NHT, 512], F16, tag="xt",
                                 name=f"xt{sc}")
                    s0 = sc * 512
                    for i, hh in enumerate(chunks):
                        nh = (chunks[i + 1] if i + 1 < len(chunks) else NHT) - hh
                        src = xt[hh * 128:(hh + nh) * 128, s0:s0 + 512]
                        nc.sync.dma_start(
                            t[:, hh:hh + nh, :],
                            src.rearrange("(h p) s -> p h s", p=128))
                    return t

                wkb = wp.tile([128, NHT, E], F16, name="wkb")
                wvb = wp.tile([128, NHT, E], F16, name="wvb")
                wqb = wp.tile([128, NHT, E], F16, name="wqb")

                def load_w(wb, wsrc, hh, nh=4):
                    src = wsrc[hh * 128:(hh + nh) * 128, :]
                    nc.sync.dma_start(
                        wb[:, hh:hh + nh, :],
                        src.rearrange("(h p) e -> p h e", p=128))

                # fine-grained first chunks so the first K-proj matmuls can
                # begin while the rest of X/W streams in
                xbig0 = xtp.tile([128, NHT, 512], F16, tag="xt", name="xt0")

                def load_x0(h0, h1):
                    nc.sync.dma_start(
                        xbig0[:, h0:h1, :],
                        xt[h0 * 128:h1 * 128, 0:512]
                        .rearrange("(h p) s -> p h s", p=128))

                load_x0(0, 1)
                load_w(wkb, wkt, 0, 1)
                load_x0(1, 2)
                load_w(wkb, wkt, 1, 1)
                load_x0(2, 4)
                load_w(wkb, wkt, 2, 2)
                load_x0(4, 8)
                load_w(wkb, wkt, 4, 4)
                nc.sync.dma_start(bk_sb[:], bk2)
                for hh in (0, 4):
                    load_w(wvb, wvt, hh)
                nc.sync.dma_start(mask_sb[:], mask2)
                nc.sync.dma_start(smask_sb[:], smask2)
                for hh in (0, 4):
                    load_w(wqb, wqt, hh)
                nc.sync.dma_start(bq_sb[:], bq2)
                # prefetch the remaining X s-chunks now: the HWDGE queue
                # streams them while sc0 computes, so later K-projections
                # never wait on just-in-time loads
                xbig_pre = {sc: load_x(sc) for sc in (1, 2, 3)}
                nc.sync.dma_start(bv_sb[:], bv2)

                p2state = {}
                # software pipeline: ctx(kt) is emitted ~CTX_LAG k-steps
                # after its scores/exp, so the in-order PE always has the
                # next scores ready while ACT/DVE computes exp.
                pending_ctx = []
                CTX_LAG = 3

                def do_ctx(et, qc, kt, ex_a, ex_b):
                    hA, hB = 2 * et, 2 * et + 1
                    key = (et, qc)
                    if key not in p2state:
                        cpA = cpp.tile([128, 260], F32, tag="cp",
                                       name=f"cpA_{et}_{qc}")
                        cpB = cpp.tile([128, 260], F32, tag="cp",
                                       name=f"cpB_{et}_{qc}")
                        p2state[key] = (cpA, cpB)
                    cpA, cpB = p2state[key]
                    # PSUM start_tensor_calc zeroing is bank-wide: only the
                    # first matmul touching each cp bank may start, or it
                    # wipes the other column-groups' accumulation.
                    for cpx, ex16, hl in ((cpA, ex_a, hA), (cpB, ex_b, hB)):
                        for j in range(4):
                            nc.tensor.matmul(
                                cpx[:, j * 65:j * 65 + 65],
                                ex16[:, j * 128:j * 128 + 128],
                                vp_t[kt][:, hl * 65:hl * 65 + 65],
                                start=(kt == 0 and j == 0),
                                stop=(kt == NKT - 1 and j == 3),
                                skip_group_check=True)

                def drain_ctx(n):
                    while len(pending_ctx) > n:
                        do_ctx(*pending_ctx.pop(0))

                def p2_kloop(et, qc, kts):
                    for kt in kts:
                        ktt = kt_t[et][kt // 4]
                        qtt = qt_t[et][qc]
                        ko = (kt % 4) * 128
                        exs = [None, None]
                        for hl2 in (0, 1):
                            sps = ssp.tile([128, QW], F32, tag="ss",
                                           name=f"sps_{et}_{qc}_{kt}_{hl2}")
                            nc.tensor.matmul(
                                sps[:],
                                ktt[hl2 * 64:hl2 * 64 + 64, ko:ko + 128],
                                qtt[hl2 * 64:hl2 * 64 + 64, :],
                                start=True, stop=True)
                            exa = exa_pool.tile([128, QW], F16, tag="exa",
                                                name=f"exa_{et}_{qc}_{kt}_{hl2}")
                            if with_sch and (kt + hl2) % 2 == 1:
                                nc.vector.tensor_scalar(
                                    exa[:].bitcast(I16),
                                    sps[:], SCH_A * 0.125,
                                    smask_sb[:, kt:kt + 1], Mult, Add)
                            else:
                                nc.scalar.activation(
                                    exa[:], sps[:], Exp,
                                    bias=mask_sb[:, kt:kt + 1], scale=0.125)
                            exs[hl2] = exa[:]
                        pending_ctx.append((et, qc, kt, exs[0], exs[1]))
                        drain_ctx(CTX_LAG)

                def p2_epilogue_ops(et, qc):
                    # returns the epilogue as single-op closures: callers
                    # interleave them one-per-k-tile into the next unit's
                    # loop so neither engine queue eats a long block and the
                    # cp banks free early for the 2-unit rotation
                    while any(p[0] == et and p[1] == qc for p in pending_ctx):
                        do_ctx(*pending_ctx.pop(0))
                    cpA, cpB = p2state.pop((et, qc))
                    og = ob_pool.tile([128, 512], F32, tag="ob",
                                      name=f"ob_{et}_{qc}")
                    ogv = og.rearrange("p (j c) -> p j c", c=128)
                    ops = []
                    for hl2, cpx in ((0, cpA), (1, cpB)):
                        cpv = cpx.rearrange("p (j c) -> p j c", c=65)
                        rc = rc_pool.tile([128, 4], F32, tag="rc",
                                          name=f"rc_{et}_{qc}_{hl2}")
                        ops.append((lambda rc=rc, cpv=cpv:
                                    nc.vector.reciprocal(rc[:],
                                                         cpv[:, :, 64:65])))
                        for j in range(4):
                            eng = nc.scalar.mul if (j + hl2) % 2 else (
                                lambda o, i, s: nc.vector.tensor_scalar_mul(o, i, s))
                            ops.append((lambda eng=eng, j=j, hl2=hl2, cpv=cpv,
                                        rc=rc:
                                        eng(ogv[:, j, hl2 * 64:hl2 * 64 + 64],
                                            cpv[:, j, 0:64], rc[:, j:j + 1])))
                    def dma():
                        od = out[qc * QW:(qc + 1) * QW,
                                 et * 128:(et + 1) * 128]
                        od = od.rearrange("(j p) c -> p j c", p=128)
                        nc.sync.dma_start(od, ogv)
                    ops.append(dma)
                    return ops

                def p2_epilogue(et, qc):
                    for op in p2_epilogue_ops(et, qc):
                        op()

                def p2_epilogue_tail(et, qc):
                    # final-unit variant: muls ordered j-major and the out
                    # DMA split in two j-halves so the first half's DMA
                    # latency hides under the second half's muls
                    while any(p[0] == et and p[1] == qc for p in pending_ctx):
                        do_ctx(*pending_ctx.pop(0))
                    cpA, cpB = p2state.pop((et, qc))
                    og = ob_pool.tile([128, 512], F32, tag="ob",
                                      name=f"ob_{et}_{qc}")
                    ogv = og.rearrange("p (j c) -> p j c", c=128)
                    cpvs, rcs = [], []
                    for hl2, cpx in ((0, cpA), (1, cpB)):
                        cpv = cpx.rearrange("p (j c) -> p j c", c=65)
                        rc = rc_pool.tile([128, 4], F32, tag="rc",
                                          name=f"rc_{et}_{qc}_{hl2}")
                        nc.vector.reciprocal(rc[:], cpv[:, :, 64:65])
                        cpvs.append(cpv)
                        rcs.append(rc)
                    for jh in (0, 1):
                        for j in (2 * jh, 2 * jh + 1):
                            for hl2 in (0, 1):
                                eng = nc.scalar.mul if (j + hl2) % 2 else (
                                    lambda o, i, s:
                                    nc.vector.tensor_scalar_mul(o, i, s))
                                eng(ogv[:, j, hl2 * 64:hl2 * 64 + 64],
                                    cpvs[hl2][:, j, 0:64],
                                    rcs[hl2][:, j:j + 1])
                        q0 = qc * QW + jh * 256
                        od = out[q0:q0 + 256, et * 128:(et + 1) * 128]
                        od = od.rearrange("(j p) c -> p j c", p=128)
                        nc.sync.dma_start(od, ogv[:, 2 * jh:2 * jh + 2, :])

                def proj_piece(wb, b_sb, dst, xb, s0, et):
                    sc = s0 // 512
                    p = ssp.tile([128, 512], F32, tag="ss",
                                 name=f"pj_{s0}_{et}")
                    for h in range(NHT):
                        nc.tensor.matmul(
                            p[:],
                            wb[:, h, et * 128:et * 128 + 128],
                            xb[:, h, :],
                            start=(h == 0), stop=(h == NHT - 1))
                    nc.vector.tensor_scalar_add(
                        dst[et][sc][:], p[:], b_sb[:, et:et + 1])

                def proj_qk(wb, b_sb, dst, xb, s0):
                    for et in range(NET):
                        proj_piece(wb, b_sb, dst, xb, s0, et)

                all_xts = {}
                for sc in range(4):  # s-chunks of 512
                    s0 = sc * 512
                    xb = xbig0 if sc == 0 else xbig_pre[sc]
                    all_xts[sc] = xb

                    proj_qk(wkb, bk_sb, kt_t, xb, s0)

                    # V s-tiles: out [s 128, e 512] (+ bias via K=1 matmul)
                    for st in range(4):
                        gst = sc * 4 + st
                        p = ssp.tile([128, 512], F32, tag="ss")
                        for h in range(NHT):
                            nc.tensor.matmul(
                                p[:],
                                xb[:, h, st * 128:st * 128 + 128],
                                wvb[:, h, :],
                                start=(h == 0),
                                stop=(not with_vbias and h == NHT - 1))
                        if with_vbias:
                            nc.tensor.matmul(p[:], ones_row[:], bv_sb[:],
                                             start=False, stop=True)
                        dstv = vp_t[gst].rearrange("p (h c) -> p h c", c=65)
                        nc.vector.tensor_copy(
                            dstv[:, :, 0:64],
                            p.rearrange("p (h c) -> p h c", c=64))

                    # Q pieces needed during phase 1 / first phase-2 unit
                    # only; all other 13 pieces are deferred, one per
                    # attention unit, as PE filler for the exp-bound units
                    if sc == 0:
                        proj_piece(wqb, bq_sb, qt_t, xb, s0, 0)
                        proj_piece(wqb, bq_sb, qt_t, xb, s0, 1)
                    elif sc == 1:
                        proj_piece(wqb, bq_sb, qt_t, xb, s0, 0)

                    # units (0,0) and (1,0): their k-tiles are ready now,
                    # so both run during phase 1 -- their exp load hides
                    # under projection PE time (ACT/DVE are otherwise idle
                    # here, and saturated during phase 2)
                    p2_kloop(0, 0, range(4 * sc, 4 * sc + 4))
                    p2_kloop(1, 0, range(4 * sc, 4 * sc + 4))

                # ---------------- Phase 2: attention (head pairs) -------
                # epilogues trail one unit behind so each unit's last ctx
                # matmuls pipeline into the next unit's scores.
                # deferred Q projections: one (sc, et) piece per unit,
                # spread across the first 8 units so each 1.7us PE burst
                # hides in a single unit's exp slack
                # (sc, et) pieces in deadline order: piece (sc, et) must be
                # emitted before unit (et, qc=sc) runs
                deferred_q = [(0, 2), (0, 3), (1, 1), (1, 2), (1, 3),
                              (2, 0), (2, 1), (2, 2), (2, 3),
                              (3, 0), (3, 1), (3, 2), (3, 3)]
                deferred_q.sort(key=lambda p: (p[1], p[0]))
                # two units finished in phase 1: burn one epilogue now so
                # the trailing-by-one rotation fits the 4 cp banks
                p2_epilogue(0, 0)
                prev_unit = (1, 0)
                ui = 0
                for et in range(NET):
                    for qc in range(S // QW):
                        if (et, qc) in ((0, 0), (1, 0)):
                            continue
                        ui += 1
                        if deferred_q and ui >= 1:
                            dsc, det = deferred_q.pop(0)
                            # must be projected before unit (0, qc=dsc) needs
                            # qt_t[*][dsc]
                            proj_piece(wqb, bq_sb, qt_t, all_xts[dsc],
                                       dsc * 512, det)
                        p2_kloop(et, qc, range(NKT))
                        p2_epilogue(*prev_unit)
                        prev_unit = (et, qc)
                drain_ctx(0)
                p2_epilogue(*prev_unit)

    nc.compile()
    return nc


def build_in_maps(inputs, with_vbias=None, with_sch=None):
    attention_mask = np.asarray(inputs["attention_mask"], dtype=np.float32)
    if with_vbias is None:
        with_vbias = bool(np.any(np.asarray(inputs["bv"], np.float32)))
    if with_sch is None:
        with_sch = bool(np.abs(attention_mask).max() < 40.0)
    suffix = f"{KERNEL_VERSION}{'b' if with_vbias else ''}{'s' if with_sch else ''}"
    hidden_states = np.asarray(inputs["hidden_states"], dtype=np.float32)
    Wq, bq = np.asarray(inputs["Wq"], np.float32), np.asarray(inputs["bq"], np.float32)
    Wk, bk = np.asarray(inputs["Wk"], np.float32), np.asarray(inputs["bk"], np.float32)
    Wv, bv = np.asarray(inputs["Wv"], np.float32), np.asarray(inputs["bv"], np.float32)

    xts = [np.ascontiguousarray(hidden_states[b].T).astype(np.float16)
           for b in range(B)]
    masks = [np.ascontiguousarray(attention_mask[b, 0, 0].reshape(NKT, 128).T)
             for b in range(B)]
    smasks = [(SCH_B + SCH_A * m).astype(np.float32) for m in masks]
    wg = []
    for g in range(2):
        rows = slice(g * E, (g + 1) * E)
        wg.append({
            "wqt": np.ascontiguousarray(Wq[rows].T).astype(np.float16),
            "wkt": np.ascontiguousarray(Wk[rows].T).astype(np.float16),
            "wvt": np.ascontiguousarray(Wv[rows].T).astype(np.float16),
            "bq2": np.ascontiguousarray(bq[rows].reshape(NET, 128).T),
            "bk2": np.ascontiguousarray(bk[rows].reshape(NET, 128).T),
            "bv2": np.ascontiguousarray(bv[rows].reshape(1, E)).astype(np.float16),
        })
    in_maps = []
    for c in range(NCORES):
        b, g = c // 2, c % 2
        in_maps.append({
            "xt": xts[b],
            f"mask2_{suffix}": masks[b],
            "smask2": smasks[b],
            **wg[g],
        })
    return in_maps


def kernel(hidden_states, attention_mask, Wq, bq, Wk, bk, Wv, bv):
    with_vbias = bool(np.any(np.asarray(bv, np.float32)))
    with_sch = bool(np.abs(np.asarray(attention_mask, np.float32)).max() < 40.0)
    ckey = ("nc", with_vbias, with_sch)
    if ckey not in _CACHE:
        _CACHE[ckey] = build_kernel(with_vbias, with_sch)
    nc = _CACHE[ckey]

    in_maps = build_in_maps(dict(
        hidden_states=hidden_states, attention_mask=attention_mask,
        Wq=Wq, bq=bq, Wk=Wk, bk=bk, Wv=Wv, bv=bv),
        with_vbias=with_vbias, with_sch=with_sch)

    trace = bool(int(os.environ.get("BASS_KERNEL_TRACE", "0")))
    res = run_bass_kernel_spmd(nc, in_maps, core_ids=list(range(NCORES)),
                               trace=trace)
    LAST_PROFILE["exec_time_ns"] = res.exec_time_ns
    LAST_PROFILE["mean_exec_time_ns"] = res.mean_exec_time_ns
    if res.instructions_and_trace is not None:
        LAST_PROFILE["trace_path"] = res.instructions_and_trace[1]

    full = np.empty((B, S, H), dtype=np.float32)
    for c in range(NCORES):
        b, g = c // 2, c % 2
        full[b][:, g * E:(g + 1) * E] = res.results[c]["out"]
    return full


# revision 61
# speedup vs baseline: 1.4505x; 1.0000x over previous
"""BERT self-attention on 8 TRN2 NeuronCores.

Problem: hidden_states [4, 2048, 1024], 16 heads x 64 dim, fp32.
Sharding: core c handles batch b = c//2 and head-group g = c%2
(8 heads = 512 embedding columns per core). Full inputs in, full
output out; slicing/transposition of inputs happens host-side here.

v14 design (per-core), HW exec ~273us (cost-model timeline) vs the
396us f32r baseline. The cost model charges a matmul only for its
moving-operand columns (1 cycle/row for fp16 at any size), so:

  All matmul operands fp16 (rel err 9.7e-3 vs the 2e-2 gate); PSUM
  accumulation fp32.
  Phase 1: Q^T/K^T [e,s] and V [s,e] projections from X^T [h,s],
           weights pre-transposed host-side, all fp16, loaded as
           [128, 4h, 512] batched DMAs (each HWDGE descriptor-gen is
           ~625ns, so few big DMAs beat many small ones; the first
           X/Wk chunks are h-pair sized so the first matmuls start
           ~4us in). Q/K biases via DVE add on the PSUM->SBUF copy;
           V bias (when nonzero) via a K=1 ones-row matmul. V' gets a
           ones column per head (softmax denominator trick) via a
           Pool-engine memset to 1.0 before the V columns land.
  Phase 2: per (head-pair et, q-chunk qc of 512), per k-tile:
           each head's scores S^T[k,q] go to their own 1-bank PSUM
           tile (4 rotating slots); exp(S*0.125 + mask_k) runs as one
           full-tile instruction per head, one head on ACT (true exp
           -> fp16) and the other on DVE (Schraudolph: fp16 bits =
           A*arg + B as int16, bitcast to fp16; ~3% rel err that
           mostly cancels between numerator and denominator), parity
           swapping per k-tile so both engines run concurrently and
           every softmax row is 50/50 exact/approx.
           ctx: queries in the PE partition dim: cp[128q, 4j x 65] +=
           ex[k, q-tile].T @ V'[k, 65] -- the fp16 moving operand is
           only 65 wide, 2x fewer PE cycles than the [65, q]
           orientation, and no PE transpose in the epilogue. ctx
           emission trails scores/exp by CTX_LAG k-tiles (software
           pipelining: the in-order PE never stalls on exp latency).
           PSUM start_tensor_calc zeroing is bank-wide, so only the
           first matmul touching a cp bank may set start=True.
           Units (0,0) and (1,0) run during phase 1 (their exp load
           hides under projection PE time); 13 of the 16 Q-projection
           (sc, et) pieces are deferred, one per attention unit in
           deadline order, as PE filler for the exp-bound units; the
           remaining X s-chunks are prefetched behind the weights.
           Epilogue (trails one unit): DVE reciprocal of col 64,
           per-partition scales split across ACT (Copy activation
           with scale AP) and DVE, one [128, 4, 128] DMA to out.
"""

import os
import numpy as np

import concourse.bass as bass
import concourse.tile as tile
from concourse import bacc, mybir
from concourse.bass_utils import run_bass_kernel_spmd

F32 = mybir.dt.float32
F16 = mybir.dt.float16
I16 = mybir.dt.int16

B, S, H = 4, 2048, 1024
NH, HD = 16, 64
NCORES = 8
E = 512          # embedding columns per core (8 heads)
NHL = 8          # heads per core
NKT = S // 128   # 16 k-tiles
NET = E // 128   # 4 e-tiles (head pairs)
NHT = H // 128   # 8 h-tiles
QW = 512         # per-head q-chunk width

# Schraudolph fast-exp constants (fp16 bits = round(A*arg + B16))
SCH_A = 1024.0 / float(np.log(2.0))
SCH_B = 1024.0 * 15 - 44.25
# Per (k-tile, head) the scores land in their own 1-bank PSUM tile and
# the exp runs as one full-tile instruction: one head on ACT (true exp),
# the other on DVE (Schraudolph bits trick), swapping per k-tile parity.
# Both engines run concurrently, halving the exp latency in the
# scores->exp->ctx chain; every softmax row is 50/50 exact/approximate.

_CACHE = {}

KERNEL_VERSION = "v14"  # bump to bust the neuron compile cache on kernel changes

LAST_PROFILE = {}


def build_kernel(with_vbias=True, with_sch=True):
    nc = bacc.Bacc("TRN2", target_bir_lowering=False, debug=False,
                   num_devices=NCORES)

    xt = nc.dram_tensor("xt", [H, S], F16, kind="ExternalInput").ap()
    wqt = nc.dram_tensor("wqt", [H, E], F16, kind="ExternalInput").ap()
    wkt = nc.dram_tensor("wkt", [H, E], F16, kind="ExternalInput").ap()
    wvt = nc.dram_tensor("wvt", [H, E], F16, kind="ExternalInput").ap()
    bq2 = nc.dram_tensor("bq2", [128, NET], F32, kind="ExternalInput").ap()
    bk2 = nc.dram_tensor("bk2", [128, NET], F32, kind="ExternalInput").ap()
    bv2 = nc.dram_tensor("bv2", [1, E], F16, kind="ExternalInput").ap()
    suffix = f"{KERNEL_VERSION}{'b' if with_vbias else ''}{'s' if with_sch else ''}"
    mask2 = nc.dram_tensor(f"mask2_{suffix}", [128, NKT], F32,
                           kind="ExternalInput").ap()
    smask2 = nc.dram_tensor("smask2", [128, NKT], F32,
                            kind="ExternalInput").ap()
    out = nc.dram_tensor("out", [S, E], F32, kind="ExternalOutput").ap()

    Exp = mybir.ActivationFunctionType.Exp
    Mult, Add = mybir.AluOpType.mult, mybir.AluOpType.add

    with tile.TileContext(nc) as tc:
        with (
            tc.tile_pool(name="persist", bufs=1) as persist,
            tc.tile_pool(name="small", bufs=1) as small,
        ):
            # persistent SBUF tensors, split per chunk so each has a
            # single producer -> exact dependencies, phases can overlap
            qt_t = [[persist.tile([128, 512], F16, name=f"qt_{et}_{sc}")
                     for sc in range(4)] for et in range(NET)]
            kt_t = [[persist.tile([128, 512], F16, name=f"kt_{et}_{sc}")
                     for sc in range(4)] for et in range(NET)]
            vp_t = [persist.tile([128, NHL * 65], F16, name=f"vp_{gst}")
                    for gst in range(NKT)]

            # small input tiles: DMAs are emitted after the big X/weight
            # loads (single HWDGE queue; each descriptor-gen is ~625ns)
            mask_sb = small.tile([128, NKT], F32)
            smask_sb = small.tile([128, NKT], F32)
            bq_sb = small.tile([128, NET], F32)
            bk_sb = small.tile([128, NET], F32)
            bv_sb = small.tile([1, E], F16)
            ones_row = small.tile([1, 128], F16)
            nc.vector.memset(ones_row[:], 1.0)

            # ones columns of V' (denominator trick): memset whole tile
            # to 1.0; the V-projection copies later overwrite cols 0:64
            # of each head's 65-block, leaving col 64 = 1.0. On the idle
            # Pool engine to keep early DVE cycles free.
            for gst in range(NKT):
                nc.gpsimd.memset(vp_t[gst][:], 1.0)

            # ---- unified pools (phases overlap at runtime) ----
            # PSUM banks: ss(proj+scores) 4 slots x 1 bank ([128,512] f32),
            #             cp 4 x 1 bank ([128, 260] f32) -> 8 total
            with (
                tc.tile_pool(name="xtp", bufs=4) as xtp,
                tc.tile_pool(name="wp", bufs=1) as wp,
                tc.tile_pool(name="ssp", bufs=4, space="PSUM") as ssp,
                tc.tile_pool(name="cpp", bufs=4, space="PSUM") as cpp,
                tc.tile_pool(name="exa", bufs=12) as exa_pool,
                tc.tile_pool(name="obp", bufs=4) as ob_pool,
                tc.tile_pool(name="rcp", bufs=8) as rc_pool,
            ):
                # ---------------- Phase 1: projections ----------------
                # batched loads: each HWDGE descriptor-gen costs ~625ns
                # regardless of size, so X / weights load as [128, 4h, 512]
                # chunks (2 DMAs per tensor), interleaved so the first
                # K-projection matmuls can start after two DMAs.
                def load_x(sc, chunks=(0, 4)):
                    t = xtp.tile([128, NHT, 512], F16, tag="xt",
                                 name=f"xt{sc}")
                    s0 = sc * 512
                    for i, hh in enumerate(chunks):
                        nh = (chunks[i + 1] if i + 1 < len(chunks) else NHT) - hh
                        src = xt[hh * 128:(hh + nh) * 128, s0:s0 + 512]
                        nc.sync.dma_start(
                            t[:, hh:hh + nh, :],
                            src.rearrange("(h p) s -> p h s", p=128))
                    return t

                wkb = wp.tile([128, NHT, E], F16, name="wkb")
                wvb = wp.tile([128, NHT, E], F16, name="wvb")
                wqb = wp.tile([128, NHT, E], F16, name="wqb")

                def load_w(wb, wsrc, hh, nh=4):
                    src = wsrc[hh * 128:(hh + nh) * 128, :]
                    nc.sync.dma_start(
                        wb[:, hh:hh + nh, :],
                        src.rearrange("(h p) e -> p h e", p=128))

                # fine-grained first chunks so the first K-proj matmuls can
                # begin while the rest of X/W streams in
                xbig0 = xtp.tile([128, NHT, 512], F16, tag="xt", name="xt0")

                def load_x0(h0, h1):
                    nc.sync.dma_start(
                        xbig0[:, h0:h1, :],
                        xt[h0 * 128:h1 * 128, 0:512]
                        .rearrange("(h p) s -> p h s", p=128))

                load_x0(0, 1)
                load_w(wkb, wkt, 0, 1)
                load_x0(1, 2)
                load_w(wkb, wkt, 1, 1)
                load_x0(2, 4)
                load_w(wkb, wkt, 2, 2)
                load_x0(4, 8)
                load_w(wkb, wkt, 4, 4)
                nc.sync.dma_start(bk_sb[:], bk2)
                for hh in (0, 4):
                    load_w(wvb, wvt, hh)
                nc.sync.dma_start(mask_sb[:], mask2)
                nc.sync.dma_start(smask_sb[:], smask2)
                for hh in (0, 4):
                    load_w(wqb, wqt, hh)
                nc.sync.dma_start(bq_sb[:], bq2)
                # prefetch the remaining X s-chunks now: the HWDGE queue
                # streams them while sc0 computes, so later K-projections
                # never wait on just-in-time loads
                xbig_pre = {sc: load_x(sc) for sc in (1, 2, 3)}
                nc.sync.dma_start(bv_sb[:], bv2)

                p2state = {}
                # software pipeline: ctx(kt) is emitted ~CTX_LAG k-steps
                # after its scores/exp, so the in-order PE always has the
                # next scores ready while ACT/DVE computes exp.
                pending_ctx = []
                CTX_LAG = 3

                def do_ctx(et, qc, kt, ex_a, ex_b):
                    hA, hB = 2 * et, 2 * et + 1
                    key = (et, qc)
                    if key not in p2state:
                        cpA = cpp.tile([128, 260], F32, tag="cp",
                                       name=f"cpA_{et}_{qc}")
                        cpB = cpp.tile([128, 260], F32, tag="cp",
                                       name=f"cpB_{et}_{qc}")
                        p2state[key] = (cpA, cpB)
                    cpA, cpB = p2state[key]
                    # PSUM start_tensor_calc zeroing is bank-wide: only the
                    # first matmul touching each cp bank may start, or it
                    # wipes the other column-groups' accumulation.
                    for cpx, ex16, hl in ((cpA, ex_a, hA), (cpB, ex_b, hB)):
                        for j in range(4):
                            nc.tensor.matmul(
                                cpx[:, j * 65:j * 65 + 65],
                                ex16[:, j * 128:j * 128 + 128],
                                vp_t[kt][:, hl * 65:hl * 65 + 65],
                                start=(kt == 0 and j == 0),
                                stop=(kt == NKT - 1 and j == 3),
                                skip_group_check=True)

                def drain_ctx(n):
                    while len(pending_ctx) > n:
                        do_ctx(*pending_ctx.pop(0))

                def p2_kloop(et, qc, kts):
                    for kt in kts:
                        ktt = kt_t[et][kt // 4]
                        qtt = qt_t[et][qc]
                        ko = (kt % 4) * 128
                        exs = [None, None]
                        for hl2 in (0, 1):
                            sps = ssp.tile([128, QW], F32, tag="ss",
                                           name=f"sps_{et}_{qc}_{kt}_{hl2}")
                            nc.tensor.matmul(
                                sps[:],
                                ktt[hl2 * 64:hl2 * 64 + 64, ko:ko + 128],
                                qtt[hl2 * 64:hl2 * 64 + 64, :],
                                start=True, stop=True)
                            exa = exa_pool.tile([128, QW], F16, tag="exa",
                                                name=f"exa_{et}_{qc}_{kt}_{hl2}")
                            if with_sch and (kt + hl2) % 2 == 1:
                                nc.vector.tensor_scalar(
                                    exa[:].bitcast(I16),
                                    sps[:], SCH_A * 0.125,
                                    smask_sb[:, kt:kt + 1], Mult, Add)
                            else:
                                nc.scalar.activation(
                                    exa[:], sps[:], Exp,
                                    bias=mask_sb[:, kt:kt + 1], scale=0.125)
                            exs[hl2] = exa[:]
                        pending_ctx.append((et, qc, kt, exs[0], exs[1]))
                        drain_ctx(CTX_LAG)

                def p2_epilogue_ops(et, qc):
                    # returns the epilogue as single-op closures: callers
                    # interleave them one-per-k-tile into the next unit's
                    # loop so neither engine queue eats a long block and the
                    # cp banks free early for the 2-unit rotation
                    while any(p[0] == et and p[1] == qc for p in pending_ctx):
                        do_ctx(*pending_ctx.pop(0))
                    cpA, cpB = p2state.pop((et, qc))
                    og = ob_pool.tile([128, 512], F32, tag="ob",
                                      name=f"ob_{et}_{qc}")
                    ogv = og.rearrange("p (j c) -> p j c", c=128)
                    ops = []
                    for hl2, cpx in ((0, cpA), (1, cpB)):
                        cpv = cpx.rearrange("p (j c) -> p j c", c=65)
                        rc = rc_pool.tile([128, 4], F32, tag="rc",
                                          name=f"rc_{et}_{qc}_{hl2}")
                        ops.append((lambda rc=rc, cpv=cpv:
                                    nc.vector.reciprocal(rc[:],
                                                         cpv[:, :, 64:65])))
                        for j in range(4):
                            eng = nc.scalar.mul if (j + hl2) % 2 else (
                                lambda o, i, s: nc.vector.tensor_scalar_mul(o, i, s))
                            ops.append((lambda eng=eng, j=j, hl2=hl2, cpv=cpv,
                                        rc=rc:
                                        eng(ogv[:, j, hl2 * 64:hl2 * 64 + 64],
                                            cpv[:, j, 0:64], rc[:, j:j + 1])))
                    def dma():
                        od = out[qc * QW:(qc + 1) * QW,
                                 et * 128:(et + 1) * 128]
                        od = od.rearrange("(j p) c -> p j c", p=128)
                        nc.sync.dma_start(od, ogv)
                    ops.append(dma)
                    return ops

                def p2_epilogue(et, qc):
                    for op in p2_epilogue_ops(et, qc):
                        op()

                def p2_epilogue_tail(et, qc):
                    # final-unit variant: muls ordered j-major and the out
                    # DMA split in two j-halves so the first half's DMA
                    # latency hides under the second half's muls
                    while any(p[0] == et and p[1] == qc for p in pending_ctx):
                        do_ctx(*pending_ctx.pop(0))
                    cpA, cpB = p2state.pop((et, qc))
                    og = ob_pool.tile([128, 512], F32, tag="ob",
                                      name=f"ob_{et}_{qc}")
                    ogv = og.rearrange("p (j c) -> p j c", c=128)
                    cpvs, rcs = [], []
                    for hl2, cpx in ((0, cpA), (1, cpB)):
                        cpv = cpx.rearrange("p (j c) -> p j c", c=65)
                        rc = rc_pool.tile([128, 4], F32, tag="rc",
                                          name=f"rc_{et}_{qc}_{hl2}")
                        nc.vector.reciprocal(rc[:], cpv[:, :, 64:65])
                        cpvs.append(cpv)
                        rcs.append(rc)
                    for jh in (0, 1):
                        for j in (2 * jh, 2 * jh + 1):
                            for hl2 in (0, 1):
                                eng = nc.scalar.mul if (j + hl2) % 2 else (
                                    lambda o, i, s:
                                    nc.vector.tensor_scalar_mul(o, i, s))
                                eng(ogv[:, j, hl2 * 64:hl2 * 64 + 64],
                                    cpvs[hl2][:, j, 0:64],
                                    rcs[hl2][:, j:j + 1])
                        q0 = qc * QW + jh * 256
                        od = out[q0:q0 + 256, et * 128:(et + 1) * 128]
                        od = od.rearrange("(j p) c -> p j c", p=128)
                        nc.sync.dma_start(od, ogv[:, 2 * jh:2 * jh + 2, :])

                def proj_piece(wb, b_sb, dst, xb, s0, et):
                    sc = s0 // 512
                    p = ssp.tile([128, 512], F32, tag="ss",
                                 name=f"pj_{s0}_{et}")
                    for h in range(NHT):
                        nc.tensor.matmul(
                            p[:],
                            wb[:, h, et * 128:et * 128 + 128],
                            xb[:, h, :],
                            start=(h == 0), stop=(h == NHT - 1))
                    nc.vector.tensor_scalar_add(
                        dst[et][sc][:], p[:], b_sb[:, et:et + 1])

                def proj_qk(wb, b_sb, dst, xb, s0):
                    for et in range(NET):
                        proj_piece(wb, b_sb, dst, xb, s0, et)

                all_xts = {}
                for sc in range(4):  # s-chunks of 512
                    s0 = sc * 512
                    xb = xbig0 if sc == 0 else xbig_pre[sc]
                    all_xts[sc] = xb

                    proj_qk(wkb, bk_sb, kt_t, xb, s0)

                    # V s-tiles: out [s 128, e 512] (+ bias via K=1 matmul)
                    for st in range(4):
                        gst = sc * 4 + st
                        p = ssp.tile([128, 512], F32, tag="ss")
                        for h in range(NHT):
                            nc.tensor.matmul(
                                p[:],
                                xb[:, h, st * 128:st * 128 + 128],
                                wvb[:, h, :],
                                start=(h == 0),
                                stop=(not with_vbias and h == NHT - 1))
                        if with_vbias:
                            nc.tensor.matmul(p[:], ones_row[:], bv_sb[:],
                                             start=False, stop=True)
                        dstv = vp_t[gst].rearrange("p (h c) -> p h c", c=65)
                        nc.vector.tensor_copy(
                            dstv[:, :, 0:64],
                            p.rearrange("p (h c) -> p h c", c=64))

                    # Q pieces needed during phase 1 / first phase-2 unit
                    # only; all other 13 pieces are deferred, one per
                    # attention unit, as PE filler for the exp-bound units
                    if sc == 0:
                        proj_piece(wqb, bq_sb, qt_t, xb, s0, 0)
                        proj_piece(wqb, bq_sb, qt_t, xb, s0, 1)
                    elif sc == 1:
                        proj_piece(wqb, bq_sb, qt_t, xb, s0, 0)

                    # units (0,0) and (1,0): their k-tiles are ready now,
                    # so both run during phase 1 -- their exp load hides
                    # under projection PE time (ACT/DVE are otherwise idle
                    # here, and saturated during phase 2)
                    for kt in range(4 * sc, 4 * sc + 4):
                        p2_kloop(0, 0, (kt,))
                        p2_kloop(1, 0, (kt,))

                # ---------------- Phase 2: attention (head pairs) -------
                # epilogues trail one unit behind so each unit's last ctx
                # matmuls pipeline into the next unit's scores.
                # deferred Q projections: one (sc, et) piece per unit,
                # spread across the first 8 units so each 1.7us PE burst
                # hides in a single unit's exp slack
                # (sc, et) pieces in deadline order: piece (sc, et) must be
                # emitted before unit (et, qc=sc) runs
                deferred_q = [(0, 2), (0, 3), (1, 1), (1, 2), (1, 3),
                              (2, 0), (2, 1), (2, 2), (2, 3),
                              (3, 0), (3, 1), (3, 2), (3, 3)]
                deferred_q.sort(key=lambda p: (p[1], p[0]))
                # two units finished in phase 1: burn one epilogue now so
                # the trailing-by-one rotation fits the 4 cp banks
                p2_epilogue(0, 0)
                prev_unit = (1, 0)
                ui = 0
                for et in range(NET):
                    for qc in range(S // QW):
                        if (et, qc) in ((0, 0), (1, 0)):
                            continue
                        ui += 1
                        if deferred_q and ui >= 1:
                            dsc, det = deferred_q.pop(0)
                            # must be projected before unit (0, qc=dsc) needs
                            # qt_t[*][dsc]
                            proj_piece(wqb, bq_sb, qt_t, all_xts[dsc],
                                       dsc * 512, det)
                        p2_kloop(et, qc, range(NKT))
                        p2_epilogue(*prev_unit)
                        prev_unit = (et, qc)
                drain_ctx(0)
                p2_epilogue(*prev_unit)

    nc.compile()
    return nc


def build_in_maps(inputs, with_vbias=None, with_sch=None):
    attention_mask = np.asarray(inputs["attention_mask"], dtype=np.float32)
    if with_vbias is None:
        with_vbias = bool(np.any(np.asarray(inputs["bv"], np.float32)))
    if with_sch is None:
        with_sch = bool(np.abs(attention_mask).max() < 40.0)
    suffix = f"{KERNEL_VERSION}{'b' if with_vbias else ''}{'s' if with_sch else ''}"
    hidden_states = np.asarray(inputs["hidden_states"], dtype=np.float32)
    Wq, bq = np.asarray(inputs["Wq"], np.float32), np.asarray(inputs["bq"], np.float32)
    Wk, bk = np.asarray(inputs["Wk"], np.float32), np.asarray(inputs["bk"], np.float32)
    Wv, bv = np.asarray(inputs["Wv"], np.float32), np.asarray(inputs["bv"], np.float32)

    xts = [np.ascontiguousarray(hidden_states[b].T).astype(np.float16)
           for b in range(B)]
    masks = [np.ascontiguousarray(attention_mask[b, 0, 0].reshape(NKT, 128).T)
             for b in range(B)]
    smasks = [(SCH_B + SCH_A * m).astype(np.float32) for m in masks]
    wg = []
    for g in range(2):
        rows = slice(g * E, (g + 1) * E)
        wg.append({
            "wqt": np.ascontiguousarray(Wq[rows].T).astype(np.float16),
            "wkt": np.ascontiguousarray(Wk[rows].T).astype(np.float16),
            "wvt": np.ascontiguousarray(Wv[rows].T).astype(np.float16),
            "bq2": np.ascontiguousarray(bq[rows].reshape(NET, 128).T),
            "bk2": np.ascontiguousarray(bk[rows].reshape(NET, 128).T),
            "bv2": np.ascontiguousarray(bv[rows].reshape(1, E)).astype(np.float16),
        })
    in_maps = []
    for c in range(NCORES):
        b, g = c // 2, c % 2
        in_maps.append({
            "xt": xts[b],
            f"mask2_{suffix}": masks[b],
            "smask2": smasks[b],
            **wg[g],
        })
    return in_maps


def kernel(hidden_states, attention_mask, Wq, bq, Wk, bk, Wv, bv):
    with_vbias = bool(np.any(np.asarray(bv, np.float32)))
    with_sch = bool(np.abs(np.asarray(attention_mask, np.float32)).max() < 40.0)
    ckey = ("nc", with_vbias, with_sch)
    if ckey not in _CACHE:
        _CACHE[ckey] = build_kernel(with_vbias, with_sch)
    nc = _CACHE[ckey]

    in_maps = build_in_maps(dict(
        hidden_states=hidden_states, attention_mask=attention_mask,
        Wq=Wq, bq=bq, Wk=Wk, bk=bk, Wv=Wv, bv=bv),
        with_vbias=with_vbias, with_sch=with_sch)

    trace = bool(int(os.environ.get("BASS_KERNEL_TRACE", "0")))
    res = run_bass_kernel_spmd(nc, in_maps, core_ids=list(range(NCORES)),
                               trace=trace)
    LAST_PROFILE["exec_time_ns"] = res.exec_time_ns
    LAST_PROFILE["mean_exec_time_ns"] = res.mean_exec_time_ns
    if res.instructions_and_trace is not None:
        LAST_PROFILE["trace_path"] = res.instructions_and_trace[1]

    full = np.empty((B, S, H), dtype=np.float32)
    for c in range(NCORES):
        b, g = c // 2, c % 2
        full[b][:, g * E:(g + 1) * E] = res.results[c]["out"]
    return full


# revision 66
# speedup vs baseline: 1.4564x; 1.0041x over previous
"""BERT self-attention on 8 TRN2 NeuronCores.

Problem: hidden_states [4, 2048, 1024], 16 heads x 64 dim, fp32.
Sharding: core c handles batch b = c//2 and head-group g = c%2
(8 heads = 512 embedding columns per core). Full inputs in, full
output out; slicing/transposition of inputs happens host-side here.

v14 design (per-core), HW exec ~273us (cost-model timeline) vs the
396us f32r baseline. The cost model charges a matmul only for its
moving-operand columns (1 cycle/row for fp16 at any size), so:

  All matmul operands fp16 (rel err 9.7e-3 vs the 2e-2 gate); PSUM
  accumulation fp32.
  Phase 1: Q^T/K^T [e,s] and V [s,e] projections from X^T [h,s],
           weights pre-transposed host-side, all fp16, loaded as
           [128, 4h, 512] batched DMAs (each HWDGE descriptor-gen is
           ~625ns, so few big DMAs beat many small ones; the first
           X/Wk chunks are h-pair sized so the first matmuls start
           ~4us in). Q/K biases via DVE add on the PSUM->SBUF copy;
           V bias (when nonzero) via a K=1 ones-row matmul. V' gets a
           ones column per head (softmax denominator trick) via a
           Pool-engine memset to 1.0 before the V columns land.
  Phase 2: per (head-pair et, q-chunk qc of 512), per k-tile:
           each head's scores S^T[k,q] go to their own 1-bank PSUM
           tile (4 rotating slots); exp(S*0.125 + mask_k) runs as one
           full-tile instruction per head, one head on ACT (true exp
           -> fp16) and the other on DVE (Schraudolph: fp16 bits =
           A*arg + B as int16, bitcast to fp16; ~3% rel err that
           mostly cancels between numerator and denominator), parity
           swapping per k-tile so both engines run concurrently and
           every softmax row is 50/50 exact/approx.
           ctx: queries in the PE partition dim: cp[128q, 4j x 65] +=
           ex[k, q-tile].T @ V'[k, 65] -- the fp16 moving operand is
           only 65 wide, 2x fewer PE cycles than the [65, q]
           orientation, and no PE transpose in the epilogue. ctx
           emission trails scores/exp by CTX_LAG k-tiles (software
           pipelining: the in-order PE never stalls on exp latency).
           PSUM start_tensor_calc zeroing is bank-wide, so only the
           first matmul touching a cp bank may set start=True.
           Units (0,0) and (1,0) run during phase 1 (their exp load
           hides under projection PE time); 13 of the 16 Q-projection
           (sc, et) pieces are deferred, one per attention unit in
           deadline order, as PE filler for the exp-bound units; the
           remaining X s-chunks are prefetched behind the weights.
           Epilogue (trails one unit): DVE reciprocal of col 64,
           per-partition scales split across ACT (Copy activation
           with scale AP) and DVE, one [128, 4, 128] DMA to out.
"""

import os
import numpy as np

import concourse.bass as bass
import concourse.tile as tile
from concourse import bacc, mybir
from concourse.bass_utils import run_bass_kernel_spmd

F32 = mybir.dt.float32
F16 = mybir.dt.float16
I16 = mybir.dt.int16

B, S, H = 4, 2048, 1024
NH, HD = 16, 64
NCORES = 8
E = 512          # embedding columns per core (8 heads)
NHL = 8          # heads per core
NKT = S // 128   # 16 k-tiles
NET = E // 128   # 4 e-tiles (head pairs)
NHT = H // 128   # 8 h-tiles
QW = 512         # per-head q-chunk width

# Schraudolph fast-exp constants (fp16 bits = round(A*arg + B16))
SCH_A = 1024.0 / float(np.log(2.0))
SCH_B = 1024.0 * 15 - 44.25
# Per (k-tile, head) the scores land in their own 1-bank PSUM tile and
# the exp runs as one full-tile instruction: one head on ACT (true exp),
# the other on DVE (Schraudolph bits trick), swapping per k-tile parity.
# Both engines run concurrently, halving the exp latency in the
# scores->exp->ctx chain; every softmax row is 50/50 exact/approximate.

_CACHE = {}

KERNEL_VERSION = "v14"  # bump to bust the neuron compile cache on kernel changes

LAST_PROFILE = {}


def build_kernel(with_vbias=True, with_sch=True):
    nc = bacc.Bacc("TRN2", target_bir_lowering=False, debug=False,
                   num_devices=NCORES)

    xt = nc.dram_tensor("xt", [H, S], F16, kind="ExternalInput").ap()
    wqt = nc.dram_tensor("wqt", [H, E], F16, kind="ExternalInput").ap()
    wkt = nc.dram_tensor("wkt", [H, E], F16, kind="ExternalInput").ap()
    wvt = nc.dram_tensor("wvt", [H, E], F16, kind="ExternalInput").ap()
    bq2 = nc.dram_tensor("bq2", [128, NET], F32, kind="ExternalInput").ap()
    bk2 = nc.dram_tensor("bk2", [128, NET], F32, kind="ExternalInput").ap()
    bv2 = nc.dram_tensor("bv2", [1, E], F16, kind="ExternalInput").ap()
    suffix = f"{KERNEL_VERSION}{'b' if with_vbias else ''}{'s' if with_sch else ''}"
    mask2 = nc.dram_tensor(f"mask2_{suffix}", [128, NKT], F32,
                           kind="ExternalInput").ap()
    smask2 = nc.dram_tensor("smask2", [128, NKT], F32,
                            kind="ExternalInput").ap()
    out = nc.dram_tensor("out", [S, E], F32, kind="ExternalOutput").ap()

    Exp = mybir.ActivationFunctionType.Exp
    Mult, Add = mybir.AluOpType.mult, mybir.AluOpType.add

    with tile.TileContext(nc) as tc:
        with (
            tc.tile_pool(name="persist", bufs=1) as persist,
            tc.tile_pool(name="small", bufs=1) as small,
        ):
            # persistent SBUF tensors, split per chunk so each has a
            # single producer -> exact dependencies, phases can overlap
            qt_t = [[persist.tile([128, 512], F16, name=f"qt_{et}_{sc}")
                     for sc in range(4)] for et in range(NET)]
            kt_t = [[persist.tile([128, 512], F16, name=f"kt_{et}_{sc}")
                     for sc in range(4)] for et in range(NET)]
            vp_t = [persist.tile([128, NHL * 65], F16, name=f"vp_{gst}")
                    for gst in range(NKT)]

            # small input tiles: DMAs are emitted after the big X/weight
            # loads (single HWDGE queue; each descriptor-gen is ~625ns)
            mask_sb = small.tile([128, NKT], F32)
            smask_sb = small.tile([128, NKT], F32)
            bq_sb = small.tile([128, NET], F32)
            bk_sb = small.tile([128, NET], F32)
            bv_sb = small.tile([1, E], F16)
            ones_row = small.tile([1, 128], F16)
            nc.vector.memset(ones_row[:], 1.0)
            warm_row = small.tile([1, 512], F16)
            nc.vector.memset(warm_row[:], 1.0)

            # ones columns of V' (denominator trick): memset whole tile
            # to 1.0; the V-projection copies later overwrite cols 0:64
            # of each head's 65-block, leaving col 64 = 1.0. On the idle
            # Pool engine to keep early DVE cycles free.
            for gst in range(NKT):
                nc.gpsimd.memset(vp_t[gst][:], 1.0)

            # ---- unified pools (phases overlap at runtime) ----
            # PSUM banks: ss(proj+scores) 4 slots x 1 bank ([128,512] f32),
            #             cp 4 x 1 bank ([128, 260] f32) -> 8 total
            with (
                tc.tile_pool(name="xtp", bufs=4) as xtp,
                tc.tile_pool(name="wp", bufs=1) as wp,
                tc.tile_pool(name="ssp", bufs=4, space="PSUM") as ssp,
                tc.tile_pool(name="cpp", bufs=4, space="PSUM") as cpp,
                tc.tile_pool(name="exa", bufs=12) as exa_pool,
                tc.tile_pool(name="obp", bufs=4) as ob_pool,
                tc.tile_pool(name="rcp", bufs=8) as rc_pool,
            ):
                # ---------------- Phase 1: projections ----------------
                # batched loads: each HWDGE descriptor-gen costs ~625ns
                # regardless of size, so X / weights load as [128, 4h, 512]
                # chunks (2 DMAs per tensor), interleaved so the first
                # K-projection matmuls can start after two DMAs.
                def load_x(sc, chunks=(0, 4)):
                    t = xtp.tile([128, NHT, 512], F16, tag="xt",
                                 name=f"xt{sc}")
                    s0 = sc * 512
                    for i, hh in enumerate(chunks):
                        nh = (chunks[i + 1] if i + 1 < len(chunks) else NHT) - hh
                        src = xt[hh * 128:(hh + nh) * 128, s0:s0 + 512]
                        nc.sync.dma_start(
                            t[:, hh:hh + nh, :],
                            src.rearrange("(h p) s -> p h s", p=128))
                    return t

                wkb = wp.tile([128, NHT, E], F16, name="wkb")
                wvb = wp.tile([128, NHT, E], F16, name="wvb")
                wqb = wp.tile([128, NHT, E], F16, name="wqb")

                def load_w(wb, wsrc, hh, nh=4):
                    src = wsrc[hh * 128:(hh + nh) * 128, :]
                    nc.sync.dma_start(
                        wb[:, hh:hh + nh, :],
                        src.rearrange("(h p) e -> p h e", p=128))

                # fine-grained first chunks so the first K-proj matmuls can
                # begin while the rest of X/W streams in
                # warm the PE p-state ramp during the startup DMA wait:
                # dummy rank-1 matmuls on memset data keep the PE busy so
                # the first real projections run at full clock
                warm_ps = ssp.tile([128, 512], F32, tag="ss", name="warm")
                for i in range(6):
                    nc.tensor.matmul(warm_ps[:], ones_row[:], warm_row[:],
                                     start=True, stop=True,
                                     skip_group_check=True)

                xbig0 = xtp.tile([128, NHT, 512], F16, tag="xt", name="xt0")

                def load_x0(h0, h1):
                    nc.sync.dma_start(
                        xbig0[:, h0:h1, :],
                        xt[h0 * 128:h1 * 128, 0:512]
                        .rearrange("(h p) s -> p h s", p=128))

                load_x0(0, 1)
                load_w(wkb, wkt, 0, 1)
                load_x0(1, 2)
                load_w(wkb, wkt, 1, 1)
                load_x0(2, 4)
                load_w(wkb, wkt, 2, 2)
                load_x0(4, 8)
                load_w(wkb, wkt, 4, 4)
                nc.sync.dma_start(bk_sb[:], bk2)
                for hh in (0, 4):
                    load_w(wvb, wvt, hh)
                nc.sync.dma_start(mask_sb[:], mask2)
                nc.sync.dma_start(smask_sb[:], smask2)
                for hh in (0, 4):
                    load_w(wqb, wqt, hh)
                nc.sync.dma_start(bq_sb[:], bq2)
                # prefetch the remaining X s-chunks now: the HWDGE queue
                # streams them while sc0 computes, so later K-projections
                # never wait on just-in-time loads
                xbig_pre = {sc: load_x(sc) for sc in (1, 2, 3)}
                nc.sync.dma_start(bv_sb[:], bv2)

                p2state = {}
                # software pipeline: ctx(kt) is emitted ~CTX_LAG k-steps
                # after its scores/exp, so the in-order PE always has the
                # next scores ready while ACT/DVE computes exp.
                pending_ctx = []
                CTX_LAG = 3

                def do_ctx(et, qc, kt, ex_a, ex_b):
                    hA, hB = 2 * et, 2 * et + 1
                    key = (et, qc)
                    if key not in p2state:
                        cpA = cpp.tile([128, 260], F32, tag="cp",
                                       name=f"cpA_{et}_{qc}")
                        cpB = cpp.tile([128, 260], F32, tag="cp",
                                       name=f"cpB_{et}_{qc}")
                        p2state[key] = (cpA, cpB)
                    cpA, cpB = p2state[key]
                    # PSUM start_tensor_calc zeroing is bank-wide: only the
                    # first matmul touching each cp bank may start, or it
                    # wipes the other column-groups' accumulation.
                    for cpx, ex16, hl in ((cpA, ex_a, hA), (cpB, ex_b, hB)):
                        for j in range(4):
                            nc.tensor.matmul(
                                cpx[:, j * 65:j * 65 + 65],
                                ex16[:, j * 128:j * 128 + 128],
                                vp_t[kt][:, hl * 65:hl * 65 + 65],
                                start=(kt == 0 and j == 0),
                                stop=(kt == NKT - 1 and j == 3),
                                skip_group_check=True)

                def drain_ctx(n):
                    while len(pending_ctx) > n:
                        do_ctx(*pending_ctx.pop(0))

                def p2_kloop(et, qc, kts):
                    for kt in kts:
                        ktt = kt_t[et][kt // 4]
                        qtt = qt_t[et][qc]
                        ko = (kt % 4) * 128
                        exs = [None, None]
                        for hl2 in (0, 1):
                            sps = ssp.tile([128, QW], F32, tag="ss",
                                           name=f"sps_{et}_{qc}_{kt}_{hl2}")
                            nc.tensor.matmul(
                                sps[:],
                                ktt[hl2 * 64:hl2 * 64 + 64, ko:ko + 128],
                                qtt[hl2 * 64:hl2 * 64 + 64, :],
                                start=True, stop=True)
                            exa = exa_pool.tile([128, QW], F16, tag="exa",
                                                name=f"exa_{et}_{qc}_{kt}_{hl2}")
                            if with_sch and (kt + hl2) % 2 == 1:
                                nc.vector.tensor_scalar(
                                    exa[:].bitcast(I16),
                                    sps[:], SCH_A * 0.125,
                                    smask_sb[:, kt:kt + 1], Mult, Add)
                            else:
                                nc.scalar.activation(
                                    exa[:], sps[:], Exp,
                                    bias=mask_sb[:, kt:kt + 1], scale=0.125)
                            exs[hl2] = exa[:]
                        pending_ctx.append((et, qc, kt, exs[0], exs[1]))
                        drain_ctx(CTX_LAG)

                def p2_epilogue_ops(et, qc):
                    # returns the epilogue as single-op closures: callers
                    # interleave them one-per-k-tile into the next unit's
                    # loop so neither engine queue eats a long block and the
                    # cp banks free early for the 2-unit rotation
                    while any(p[0] == et and p[1] == qc for p in pending_ctx):
                        do_ctx(*pending_ctx.pop(0))
                    cpA, cpB = p2state.pop((et, qc))
                    og = ob_pool.tile([128, 512], F32, tag="ob",
                                      name=f"ob_{et}_{qc}")
                    ogv = og.rearrange("p (j c) -> p j c", c=128)
                    ops = []
                    for hl2, cpx in ((0, cpA), (1, cpB)):
                        cpv = cpx.rearrange("p (j c) -> p j c", c=65)
                        rc = rc_pool.tile([128, 4], F32, tag="rc",
                                          name=f"rc_{et}_{qc}_{hl2}")
                        ops.append((lambda rc=rc, cpv=cpv:
                                    nc.vector.reciprocal(rc[:],
                                                         cpv[:, :, 64:65])))
                        for j in range(4):
                            eng = nc.scalar.mul if (j + hl2) % 2 else (
                                lambda o, i, s: nc.vector.tensor_scalar_mul(o, i, s))
                            ops.append((lambda eng=eng, j=j, hl2=hl2, cpv=cpv,
                                        rc=rc:
                                        eng(ogv[:, j, hl2 * 64:hl2 * 64 + 64],
                                            cpv[:, j, 0:64], rc[:, j:j + 1])))
                    def dma():
                        od = out[qc * QW:(qc + 1) * QW,
                                 et * 128:(et + 1) * 128]
                        od = od.rearrange("(j p) c -> p j c", p=128)
                        nc.sync.dma_start(od, ogv)
                    ops.append(dma)
                    return ops

                def p2_epilogue(et, qc):
                    for op in p2_epilogue_ops(et, qc):
                        op()

                def p2_epilogue_tail(et, qc):
                    # final-unit variant: muls ordered j-major and the out
                    # DMA split in two j-halves so the first half's DMA
                    # latency hides under the second half's muls
                    while any(p[0] == et and p[1] == qc for p in pending_ctx):
                        do_ctx(*pending_ctx.pop(0))
                    cpA, cpB = p2state.pop((et, qc))
                    og = ob_pool.tile([128, 512], F32, tag="ob",
                                      name=f"ob_{et}_{qc}")
                    ogv = og.rearrange("p (j c) -> p j c", c=128)
                    cpvs, rcs = [], []
                    for hl2, cpx in ((0, cpA), (1, cpB)):
                        cpv = cpx.rearrange("p (j c) -> p j c", c=65)
                        rc = rc_pool.tile([128, 4], F32, tag="rc",
                                          name=f"rc_{et}_{qc}_{hl2}")
                        nc.vector.reciprocal(rc[:], cpv[:, :, 64:65])
                        cpvs.append(cpv)
                        rcs.append(rc)
                    for jh in (0, 1):
                        for j in (2 * jh, 2 * jh + 1):
                            for hl2 in (0, 1):
                                eng = nc.scalar.mul if (j + hl2) % 2 else (
                                    lambda o, i, s:
                                    nc.vector.tensor_scalar_mul(o, i, s))
                                eng(ogv[:, j, hl2 * 64:hl2 * 64 + 64],
                                    cpvs[hl2][:, j, 0:64],
                                    rcs[hl2][:, j:j + 1])
                        q0 = qc * QW + jh * 256
                        od = out[q0:q0 + 256, et * 128:(et + 1) * 128]
                        od = od.rearrange("(j p) c -> p j c", p=128)
                        nc.sync.dma_start(od, ogv[:, 2 * jh:2 * jh + 2, :])

                def proj_piece(wb, b_sb, dst, xb, s0, et):
                    sc = s0 // 512
                    p = ssp.tile([128, 512], F32, tag="ss",
                                 name=f"pj_{s0}_{et}")
                    for h in range(NHT):
                        nc.tensor.matmul(
                            p[:],
                            wb[:, h, et * 128:et * 128 + 128],
                            xb[:, h, :],
                            start=(h == 0), stop=(h == NHT - 1))
                    nc.vector.tensor_scalar_add(
                        dst[et][sc][:], p[:], b_sb[:, et:et + 1])

                def proj_qk(wb, b_sb, dst, xb, s0):
                    for et in range(NET):
                        proj_piece(wb, b_sb, dst, xb, s0, et)

                all_xts = {}
                for sc in range(4):  # s-chunks of 512
                    s0 = sc * 512
                    xb = xbig0 if sc == 0 else xbig_pre[sc]
                    all_xts[sc] = xb

                    proj_qk(wkb, bk_sb, kt_t, xb, s0)

                    # V s-tiles: out [s 128, e 512] (+ bias via K=1 matmul)
                    for st in range(4):
                        gst = sc * 4 + st
                        p = ssp.tile([128, 512], F32, tag="ss")
                        for h in range(NHT):
                            nc.tensor.matmul(
                                p[:],
                                xb[:, h, st * 128:st * 128 + 128],
                                wvb[:, h, :],
                                start=(h == 0),
                                stop=(not with_vbias and h == NHT - 1))
                        if with_vbias:
                            nc.tensor.matmul(p[:], ones_row[:], bv_sb[:],
                                             start=False, stop=True)
                        dstv = vp_t[gst].rearrange("p (h c) -> p h c", c=65)
                        nc.vector.tensor_copy(
                            dstv[:, :, 0:64],
                            p.rearrange("p (h c) -> p h c", c=64))

                    # Q pieces needed during phase 1 / first phase-2 unit
                    # only; all other 13 pieces are deferred, one per
                    # attention unit, as PE filler for the exp-bound units
                    if sc == 0:
                        proj_piece(wqb, bq_sb, qt_t, xb, s0, 0)
                        proj_piece(wqb, bq_sb, qt_t, xb, s0, 1)
                    elif sc == 1:
                        proj_piece(wqb, bq_sb, qt_t, xb, s0, 0)

                    # units (0,0) and (1,0): their k-tiles are ready now,
                    # so both run during phase 1 -- their exp load hides
                    # under projection PE time (ACT/DVE are otherwise idle
                    # here, and saturated during phase 2)
                    for kt in range(4 * sc, 4 * sc + 4):
                        p2_kloop(0, 0, (kt,))
                        p2_kloop(1, 0, (kt,))

                # ---------------- Phase 2: attention (head pairs) -------
                # epilogues trail one unit behind so each unit's last ctx
                # matmuls pipeline into the next unit's scores.
                # deferred Q projections: one (sc, et) piece per unit,
                # spread across the first 8 units so each 1.7us PE burst
                # hides in a single unit's exp slack
                # (sc, et) pieces in deadline order: piece (sc, et) must be
                # emitted before unit (et, qc=sc) runs
                deferred_q = [(0, 2), (0, 3), (1, 1), (1, 2), (1, 3),
                              (2, 0), (2, 1), (2, 2), (2, 3),
                              (3, 0), (3, 1), (3, 2), (3, 3)]
                deferred_q.sort(key=lambda p: (p[1], p[0]))
                # two units finished in phase 1: burn one epilogue now so
                # the trailing-by-one rotation fits the 4 cp banks
                p2_epilogue(0, 0)
                prev_unit = (1, 0)
                ui = 0
                for et in range(NET):
                    for qc in range(S // QW):
                        if (et, qc) in ((0, 0), (1, 0)):
                            continue
                        ui += 1
                        if deferred_q and ui >= 1:
                            dsc, det = deferred_q.pop(0)
                            # must be projected before unit (0, qc=dsc) needs
                            # qt_t[*][dsc]
                            proj_piece(wqb, bq_sb, qt_t, all_xts[dsc],
                                       dsc * 512, det)
                        p2_kloop(et, qc, range(NKT))
                        p2_epilogue(*prev_unit)
                        prev_unit = (et, qc)
                drain_ctx(0)
                # final unit: keep the whole epilogue on the DVE so the
                # last mul chain pipelines without cross-engine sem hops
                (et, qc) = prev_unit
                while any(p[0] == et and p[1] == qc for p in pending_ctx):
                    do_ctx(*pending_ctx.pop(0))
                cpA, cpB = p2state.pop((et, qc))
                og = ob_pool.tile([128, 512], F32, tag="ob", name="ob_last")
                ogv = og.rearrange("p (j c) -> p j c", c=128)
                for hl2, cpx in ((0, cpA), (1, cpB)):
                    cpv = cpx.rearrange("p (j c) -> p j c", c=65)
                    rc = rc_pool.tile([128, 4], F32, tag="rc",
                                      name=f"rc_last_{hl2}")
                    nc.vector.reciprocal(rc[:], cpv[:, :, 64:65])
                    for j in range(4):
                        nc.vector.tensor_scalar_mul(
                            ogv[:, j, hl2 * 64:hl2 * 64 + 64],
                            cpv[:, j, 0:64], rc[:, j:j + 1])
                od = out[qc * QW:(qc + 1) * QW, et * 128:(et + 1) * 128]
                od = od.rearrange("(j p) c -> p j c", p=128)
                nc.sync.dma_start(od, ogv)

    nc.compile()
    return nc


def build_in_maps(inputs, with_vbias=None, with_sch=None):
    attention_mask = np.asarray(inputs["attention_mask"], dtype=np.float32)
    if with_vbias is None:
        with_vbias = bool(np.any(np.asarray(inputs["bv"], np.float32)))
    if with_sch is None:
        with_sch = bool(np.abs(attention_mask).max() < 40.0)
    suffix = f"{KERNEL_VERSION}{'b' if with_vbias else ''}{'s' if with_sch else ''}"
    hidden_states = np.asarray(inputs["hidden_states"], dtype=np.float32)
    Wq, bq = np.asarray(inputs["Wq"], np.float32), np.asarray(inputs["bq"], np.float32)
    Wk, bk = np.asarray(inputs["Wk"], np.float32), np.asarray(inputs["bk"], np.float32)
    Wv, bv = np.asarray(inputs["Wv"], np.float32), np.asarray(inputs["bv"], np.float32)

    xts = [np.ascontiguousarray(hidden_states[b].T).astype(np.float16)
           for b in range(B)]
    masks = [np.ascontiguousarray(attention_mask[b, 0, 0].reshape(NKT, 128).T)
             for b in range(B)]
    smasks = [(SCH_B + SCH_A * m).astype(np.float32) for m in masks]
    wg = []
    for g in range(2):
        rows = slice(g * E, (g + 1) * E)
        wg.append({
            "wqt": np.ascontiguousarray(Wq[rows].T).astype(np.float16),
            "wkt": np.ascontiguousarray(Wk[rows].T).astype(np.float16),
            "wvt": np.ascontiguousarray(Wv[rows].T).astype(np.float16),
            "bq2": np.ascontiguousarray(bq[rows].reshape(NET, 128).T),
            "bk2": np.ascontiguousarray(bk[rows].reshape(NET, 128).T),
            "bv2": np.ascontiguousarray(bv[rows].reshape(1, E)).astype(np.float16),
        })
    in_maps = []
    for c in range(NCORES):
        b, g = c // 2, c % 2
        in_maps.append({
            "xt": xts[b],
            f"mask2_{suffix}": masks[b],
            "smask2": smasks[b],
            **wg[g],
        })
    return in_maps


def kernel(hidden_states, attention_mask, Wq, bq, Wk, bk, Wv, bv):
    with_vbias = bool(np.any(np.asarray(bv, np.float32)))
    with_sch = bool(np.abs(np.asarray(attention_mask, np.float32)).max() < 40.0)
    ckey = ("nc", with_vbias, with_sch)
    if ckey not in _CACHE:
        _CACHE[ckey] = build_kernel(with_vbias, with_sch)
    nc = _CACHE[ckey]

    in_maps = build_in_maps(dict(
        hidden_states=hidden_states, attention_mask=attention_mask,
        Wq=Wq, bq=bq, Wk=Wk, bk=bk, Wv=Wv, bv=bv),
        with_vbias=with_vbias, with_sch=with_sch)

    trace = bool(int(os.environ.get("BASS_KERNEL_TRACE", "0")))
    res = run_bass_kernel_spmd(nc, in_maps, core_ids=list(range(NCORES)),
                               trace=trace)
    LAST_PROFILE["exec_time_ns"] = res.exec_time_ns
    LAST_PROFILE["mean_exec_time_ns"] = res.mean_exec_time_ns
    if res.instructions_and_trace is not None:
        LAST_PROFILE["trace_path"] = res.instructions_and_trace[1]

    full = np.empty((B, S, H), dtype=np.float32)
    for c in range(NCORES):
        b, g = c // 2, c % 2
        full[b][:, g * E:(g + 1) * E] = res.results[c]["out"]
    return full
